# revision 69
# baseline (speedup 1.0000x reference)
"""GatedDeltaNet block kernel for 8 Trainium2 cores (Bass/Tile).

Sharding: DP2 (batch) x TP4 (heads / MLP-inter). Core c: group g=c//4 runs
batch g; member m=c%4 owns heads [8m,8m+8), q/k cols [384m,..), v/g cols
[768m,..), INTER [1408m,..). One on-device AllReduce per 4-core group after
o_proj; final down-proj partials summed on the host.

Per-core dataflow:
  A: x -> rmsnorm (token-major) -> PE-transpose -> hT [2048,1024] f32r (SBUF)
  B: fp32r projections off hT; q/k feature-major (heads padded to 64 rows)
     -> conv+silu+l2norm -> bf16 SBUF qS/kS; v -> conv+silu -> bf16 SBUF vS;
     gate token-major direct (silu at evict) -> SBUF; a/b -> SBUF
  C: chunked gated delta rule (C=128, UT transform via log-doubling inverse)
     with bf16 matmul operands / fp32 psum+state; fused DVE ops; writes
     normed+gated o to DRAM (f32r)
  D: o_proj token-major partial -> AllReduce (group of 4)
  E: h2 = x + o_sum; rmsnorm -> transpose -> ffT (reuses hT slot); MLP f32r;
     y = W2_partial + 0.25*h2  (host sums the 4 partials per group)
"""
import sys
sys.path.insert(0, '/opt/trn_rl_repo')
import numpy as np

import concourse.bass as bass
import concourse.bacc as bacc
import concourse.mybir as mybir
import concourse.tile as tile
from concourse.bass_isa import ReduceOp
from concourse.bass_utils import run_bass_kernel_spmd

F32 = mybir.dt.float32
F32R = mybir.dt.float32r
BF16 = mybir.dt.bfloat16
AF = mybir.ActivationFunctionType
OP = mybir.AluOpType

B, T, D = 2, 1024, 2048
H, DK, DV = 32, 48, 96
HP = 8
QKP = 512
VD_C = 768
VP = 1024
INT_C = 1408
C = 128
NCHUNK = T // C
KT = D // 128
NTOK = T // 128

_cache = {}
import os
PHASES = os.environ.get("DN_PHASES", "ABCDE")
NCH = int(os.environ.get("DN_NCHUNK", str(T // C)))


class _SkipRest(Exception):
    pass


def _build(n_cores=8):
    groups = [[0, 1, 2, 3], [4, 5, 6, 7]] if n_cores == 8 else [[0]]
    nc = bacc.Bacc("TRN2", target_bir_lowering=False, debug=False, num_devices=n_cores)

    x_d = nc.dram_tensor("x", [T, D], F32, kind="ExternalInput")
    wq_d = nc.dram_tensor("wq", [D, QKP], BF16, kind="ExternalInput")
    wk_d = nc.dram_tensor("wk", [D, QKP], BF16, kind="ExternalInput")
    wv_d = nc.dram_tensor("wv", [D, VP], BF16, kind="ExternalInput")
    wg_d = nc.dram_tensor("wg", [D, VD_C], BF16, kind="ExternalInput")
    wab_d = nc.dram_tensor("wab", [D, 16], F32, kind="ExternalInput")
    cq_d = nc.dram_tensor("cq", [QKP, 4], F32, kind="ExternalInput")
    ck_d = nc.dram_tensor("ck", [QKP, 4], F32, kind="ExternalInput")
    cv_d = nc.dram_tensor("cv", [VP, 4], F32, kind="ExternalInput")
    dtb_d = nc.dram_tensor("dtb", [1, HP], F32, kind="ExternalInput")
    nega_d = nc.dram_tensor("nega", [1, HP], F32, kind="ExternalInput")
    wo_d = nc.dram_tensor("wo", [VD_C, D], BF16, kind="ExternalInput")
    w1_d = nc.dram_tensor("w1", [D, INT_C], BF16, kind="ExternalInput")
    w3_d = nc.dram_tensor("w3", [D, INT_C], BF16, kind="ExternalInput")
    w2_d = nc.dram_tensor("w2", [INT_C, D], BF16, kind="ExternalInput")
    y_d = nc.dram_tensor("y", [T, D], F32, kind="ExternalOutput")

    idn_c = nc.inline_tensor(np.eye(128, dtype=np.float32), "idn_c")
    ones = np.ones((128, 128), np.float32)
    cum_c = nc.inline_tensor(np.triu(ones).copy(), "cum_c")
    mst_c = nc.inline_tensor(np.triu(ones, 1).copy(), "mst_c")
    negl_c = nc.inline_tensor((np.tril(ones, -1) * -1e30).copy(), "negl_c")
    sel_np = np.zeros((HP, 512), np.float32)
    for j in range(4):
        sel_np[2 * j, 128 * j:128 * j + 48] = 1.0
        sel_np[2 * j + 1, 128 * j + 64:128 * j + 112] = 1.0
    sel_c = nc.inline_tensor(sel_np, "sel_c")
    on48_np = np.zeros((128, 2), np.float32)
    on48_np[0:48, 0] = 1.0
    on48_np[64:112, 1] = 1.0
    on48_c = nc.inline_tensor(on48_np, "on48_c")
    oneh_np = np.zeros((HP, HP * 128), np.float32)
    for h in range(HP):
        oneh_np[h, 128 * h:128 * h + 128] = 1.0
    oneh_c = nc.inline_tensor(oneh_np, "oneh_c")

    with tile.TileContext(nc) as tc:
      try:
        cpool = tc.alloc_tile_pool(name="consts", bufs=1)
        big = tc.alloc_tile_pool(name="big", bufs=1)
        pg = tc.alloc_tile_pool(name="pg", bufs=1)
        wp = tc.alloc_tile_pool(name="wp", bufs=6)
        oSp = tc.alloc_tile_pool(name="oSp", bufs=1)
        qkvp = tc.alloc_tile_pool(name="qkvp", bufs=1)
        dram = tc.alloc_tile_pool(name="dram", bufs=1, space="DRAM")

        idn = cpool.tile([128, 128], F32)
        idh = cpool.tile([128, 128], BF16)
        cum = cpool.tile([128, 128], F32)
        mst = cpool.tile([128, 128], BF16)
        negl = cpool.tile([128, 128], F32)
        sel = cpool.tile([HP, 512], F32)
        on48 = cpool.tile([128, 2], F32)
        oneh = cpool.tile([HP, HP * 128], F32)
        for t_, s_ in [(idn, idn_c), (cum, cum_c), (sel, sel_c),
                       (negl, negl_c), (oneh, oneh_c)]:
            nc.sync.dma_start(t_[:], s_[:])
        nc.gpsimd.dma_start(mst[:], mst_c[:])
        nc.sync.dma_start(on48[:], on48_c[:])
        nc.vector.tensor_copy(idh[:], idn[:])
        eps1 = cpool.tile([128, 1], F32)
        nc.vector.memset(eps1[:], 1e-5)
        epsq = cpool.tile([128, 1], F32)
        nc.vector.memset(epsq[:], 48e-6)
        epsk = cpool.tile([128, 1], F32)
        nc.vector.memset(epsk[:], 1e-6)
        epsg = cpool.tile([128, 1], F32)
        nc.vector.memset(epsg[:], 1e-5)
        dtb_r = cpool.tile([1, HP], F32)
        nega_r = cpool.tile([1, HP], F32)
        nc.sync.dma_start(dtb_r[:], dtb_d[:])
        nc.sync.dma_start(nega_r[:], nega_d[:])
        dtb_bc = cpool.tile([128, HP], F32)
        nega_bc = cpool.tile([128, HP], F32)
        nc.gpsimd.partition_broadcast(dtb_bc[:], dtb_r[:])
        nc.gpsimd.partition_broadcast(nega_bc[:], nega_r[:])
        cqw = cpool.tile([128, 16], F32)
        ckw = cpool.tile([128, 16], F32)
        cvw = cpool.tile([128, 32], F32)
        for j in range(4):
            nc.sync.dma_start(cqw[:, 4 * j:4 * j + 4], cq_d[128 * j:128 * j + 128, :])
            nc.sync.dma_start(ckw[:, 4 * j:4 * j + 4], ck_d[128 * j:128 * j + 128, :])
        for j in range(8):
            nc.sync.dma_start(cvw[:, 4 * j:4 * j + 4], cv_d[128 * j:128 * j + 128, :])
        wab_s = cpool.tile([128, 16 * KT], F32)
        for k in range(KT):
            nc.sync.dma_start(wab_s[:, 16 * k:16 * k + 16], wab_d[128 * k:128 * k + 128, :])
        ab_fm = cpool.tile([16, 1024], F32)

        hT = big.tile([128, KT * 1024], BF16)
        g_tok = pg.tile([128, NTOK * VD_C], BF16, tag="gtok")
        # o kept SBUF-resident, per-head 128-col token blocks (rows 0:96 used)
        oS = oSp.tile([128, HP * 1024], BF16, tag="oS")
        qS = qkvp.tile([128, 4 * 1024], BF16, tag="qS")
        kS = qkvp.tile([128, 4 * 1024], BF16, tag="kS")
        vS = qkvp.tile([128, 8 * 1024], BF16, tag="vS")
        nc.vector.memset(qS[:], 0.0)
        nc.vector.memset(kS[:], 0.0)

        o_in = dram.tile([T, D], BF16)
        o_out = dram.tile([T, D], BF16)
        h2_scr = dram.tile([T, D], F32)

        # ============ Phase A ============
        psAB = tc.alloc_tile_pool(name="psAB", bufs=8, space="PSUM")

        def pst(p=128, f=512):
            return psAB.tile([p, f], F32, tag="ps", name="pst")

        stA = tc.alloc_tile_pool(name="stA", bufs=3)
        stA2 = tc.alloc_tile_pool(name="stA2", bufs=4)
        for i in range(NTOK):
            xa = stA.tile([128, D], F32, tag="x2k")
            nc.sync.dma_start(xa[:], x_d[128 * i:128 * i + 128, :])
            sq = stA.tile([128, D], F32, tag="x2k")
            rcol = stA.tile([128, 1], F32, tag="rcol")
            nc.vector.tensor_mul(sq[:], xa[:], xa[:])
            nc.vector.tensor_reduce(rcol[:], sq[:], mybir.AxisListType.X, OP.add)
            nc.scalar.activation(rcol[:], rcol[:], AF.Sqrt, bias=eps1[:], scale=1.0 / D)
            nc.vector.reciprocal(rcol[:], rcol[:])
            nc.vector.tensor_scalar_mul(xa[:], xa[:], rcol[:])
            p_abi = pst(16, 128)
            for k in range(KT):
                pt = pst(128, 128)
                nc.tensor.transpose(pt[:], xa[:, 128 * k:128 * k + 128], idn[:])
                st32 = stA2.tile([128, 128], F32, tag="st32")
                if k % 2 == 0:
                    nc.scalar.copy(st32[:], pt[:])
                else:
                    nc.vector.tensor_copy(st32[:], pt[:])
                nc.gpsimd.tensor_copy(hT[:, 1024 * k + 128 * i:1024 * k + 128 * i + 128], st32[:])
                nc.tensor.matmul(p_abi[:], wab_s[:, 16 * k:16 * k + 16], st32[:],
                                 start=(k == 0), stop=(k == KT - 1))
            nc.vector.tensor_copy(ab_fm[:, 128 * i:128 * i + 128], p_abi[:])
        stA2.release()
        stA.release()

        # ============ Phase B ============
        if "B" not in PHASES:
            raise _SkipRest()
        pb = tc.alloc_tile_pool(name="pb", bufs=6)

        def conv_silu(pre, cw, j, out_ap):
            acc = pb.tile([128, 1024], F32, tag="s1k")
            nc.scalar.activation(acc[:], pre[:], AF.Copy, scale=cw[:, 4 * j + 3:4 * j + 4])
            for s in (1, 2, 3):
                nc.vector.scalar_tensor_tensor(
                    acc[:, s:1024], pre[:, 0:1024 - s], cw[:, 4 * j + 3 - s:4 * j + 4 - s],
                    acc[:, s:1024], op0=OP.mult, op1=OP.add)
            nc.scalar.activation(out_ap, acc[:], AF.Silu)

        def qkv_pass(w_dram, outS, cw, eps_col, mult, do_l2, jbase, wcol0):
            # one pass: 4 feature blocks, k-outer, 8 psums, 1 wide DMA per k
            pps = [[pst() for n in range(2)] for j in range(4)]
            for k in range(KT):
                wt = wp.tile([128, 512], BF16, tag="wwide")
                nc.sync.dma_start(
                    wt[:], w_dram[128 * k:128 * k + 128, wcol0:wcol0 + 512])
                for j in range(4):
                    for n in range(2):
                        nc.tensor.matmul(
                            pps[j][n][:], wt[:, 128 * j:128 * j + 128],
                            hT[:, 1024 * k + 512 * n:1024 * k + 512 * n + 512],
                            start=(k == 0), stop=(k == KT - 1))
            for j in range(4):
                jj = jbase + j
                pre = pb.tile([128, 1024], F32, tag="s1k")
                for n in range(2):
                    nc.vector.tensor_copy(pre[:, 512 * n:512 * n + 512], pps[j][n][:])
                if not do_l2:
                    conv_silu(pre, cw, jj, outS[:, 1024 * jj:1024 * jj + 1024])
                    continue
                blk = pb.tile([128, 1024], F32, tag="s1k")
                conv_silu(pre, cw, jj, blk[:])
                sq = pb.tile([128, 1024], F32, tag="s1k")
                nc.gpsimd.tensor_mul(sq[:], blk[:], blk[:])
                for hh, rh in ((0, 0), (1, 64)):
                    srow = pb.tile([1, 1024], F32, tag="srow")
                    for n2 in range(2):
                        p_ssq = pst(1, 512)
                        nc.tensor.matmul(
                            p_ssq[:], on48[:, hh:hh + 1], sq[:, 512 * n2:512 * n2 + 512],
                            start=True, stop=True)
                        nc.scalar.activation(srow[:, 512 * n2:512 * n2 + 512], p_ssq[:],
                                             AF.Sqrt, bias=eps_col[0:1, :], scale=mult)
                    sbc = pb.tile([128, 1024], F32, tag="s1k")
                    nc.gpsimd.partition_broadcast(sbc[:], srow[:])
                    nc.vector.reciprocal(sbc[rh:rh + 48, :], sbc[rh:rh + 48, :])
                    nc.vector.tensor_mul(
                        outS[rh:rh + 48, 1024 * jj:1024 * jj + 1024],
                        blk[rh:rh + 48, :], sbc[rh:rh + 48, :])

        qkv_pass(wq_d, qS, cqw, epsq, 48.0, True, 0, 0)
        qkv_pass(wk_d, kS, ckw, epsk, 1.0, True, 0, 0)
        qkv_pass(wv_d, vS, cvw, None, None, False, 0, 0)
        qkv_pass(wv_d, vS, cvw, None, None, False, 4, 512)

        # gate token-major
        for n in range(2):
            pgs = [pst(128, 384) for _ in range(NTOK)]
            for k in range(KT):
                wt = wp.tile([128, 384], BF16, tag="wg384")
                nc.sync.dma_start(
                    wt[:], wg_d[128 * k:128 * k + 128, 384 * n:384 * n + 384])
                for i in range(NTOK):
                    nc.tensor.matmul(
                        pgs[i][:], hT[:, 1024 * k + 128 * i:1024 * k + 128 * i + 128], wt[:],
                        start=(k == 0), stop=(k == KT - 1))
            for i in range(NTOK):
                nc.scalar.activation(
                    g_tok[:, VD_C * i + 384 * n:VD_C * i + 384 * n + 384], pgs[i][:], AF.Silu)
        pb.release()
        psAB.release()

        # ============ Phase C (+ interleaved Phase D o_proj) ============
        if "C" not in PHASES:
            raise _SkipRest()
        dbf = tc.alloc_tile_pool(name="dbf", bufs=26)
        dxp = tc.alloc_tile_pool(name="dxp", bufs=10)
        dsl = tc.alloc_tile_pool(name="dsl", bufs=6)
        df32 = tc.alloc_tile_pool(name="df32", bufs=5)
        dp2 = tc.alloc_tile_pool(name="dp2", bufs=2)
        dp3 = tc.alloc_tile_pool(name="dp3", bufs=6)
        spool = tc.alloc_tile_pool(name="spool", bufs=2)
        wp2 = tc.alloc_tile_pool(name="wp2", bufs=9)
        pd = tc.alloc_tile_pool(name="pd", bufs=3)
        psC = tc.alloc_tile_pool(name="psC", bufs=5, space="PSUM")
        psCh = tc.alloc_tile_pool(name="psCh", bufs=3, space="PSUM")

        def cpst():
            return psC.tile([128, 512], F32, tag="c", name="cpst")

        def cpsth():
            return psCh.tile([128, 1024], BF16, tag="ch", name="cpsth")

        def b128():
            return dbf.tile([128, 128], BF16, tag="b128", name="b128")

        S_cur = spool.tile([128, 4 * DV], F32, tag="s", name="s")
        nc.vector.memset(S_cur[:], 0.0)
        do_d = "D" in PHASES

        for ci in range(NCH):
            cs = slice(128 * ci, 128 * ci + 128)
            # --- per-chunk decay/beta prep (f32); pPrep bank: ab@0, bcum@128, bT@256, ebc4@384
            pPrep = cpst()
            nc.tensor.transpose(pPrep[:, 0:16], ab_fm[:, cs], idn[0:16, 0:16])
            gt = dp2.tile([128, HP], F32, tag="gt")
            nc.vector.tensor_add(gt[:], pPrep[:, 0:HP], dtb_bc[:])
            nc.scalar.activation(gt[:], gt[:], AF.Exp)
            nc.vector.tensor_scalar_add(gt[:], gt[:], 1.0)
            nc.scalar.activation(gt[:], gt[:], AF.Ln)
            nc.vector.tensor_mul(gt[:], gt[:], nega_bc[:])
            beta = dp2.tile([128, HP], F32, tag="beta")
            nc.scalar.activation(beta[:], pPrep[:, HP:16], AF.Sigmoid)
            nbeta = dp2.tile([128, HP], F32, tag="nbeta")
            nc.vector.tensor_scalar_mul(nbeta[:], beta[:], -1.0)
            nc.tensor.matmul(pPrep[:, 128:128 + HP], cum[:], gt[:], start=True, stop=True)
            bcum = dp2.tile([128, HP], F32, tag="bcum")
            nc.vector.tensor_copy(bcum[:], pPrep[:, 128:128 + HP])
            lam = dp2.tile([128, HP], F32, tag="lam")
            nc.scalar.activation(lam[:], pPrep[:, 128:128 + HP], AF.Exp)
            nlam = dp2.tile([128, HP], F32, tag="nlam")
            nc.vector.tensor_scalar_mul(nlam[:], lam[:], -1.0)
            nc.tensor.transpose(pPrep[0:HP, 256:384], bcum[:], idn[:])
            b_fm = dp2.tile([HP, 128], F32, tag="bfm")
            nc.vector.tensor_copy(b_fm[:], pPrep[0:HP, 256:384])
            ebc = dp2.tile([HP, 1], F32, tag="ebc")
            nc.scalar.activation(ebc[:], b_fm[:, 127:128], AF.Exp)
            for j in range(4):
                nc.tensor.matmul(pPrep[:, 384 + j:385 + j], sel[:, 128 * j:128 * j + 128],
                                 ebc[:], start=True, stop=True)
            ebc4 = dp2.tile([128, 4], F32, tag="ebc4")
            nc.vector.tensor_copy(ebc4[:], pPrep[:, 384:388])

            # v token-major: pack all 8 heads' transposes in one bf16 bank
            pVt = cpsth()
            for h in range(HP):
                nc.tensor.transpose(pVt[:, DV * h:DV * h + DV],
                                    vS[0:DV, 1024 * h + 128 * ci:1024 * h + 128 * ci + 128],
                                    idh[0:DV, 0:DV])
            v_tok = dp2.tile([128, HP * DV], F32, tag="vtok")
            nc.vector.tensor_copy(v_tok[:], pVt[:, 0:HP * DV])

            # k token-major (for kw), packed
            pKt = cpsth()
            for j in range(4):
                nc.tensor.transpose(pKt[:, 128 * j:128 * j + 128],
                                    kS[:, 1024 * j + 128 * ci:1024 * j + 128 * ci + 128], idh[:])
            pXX = cpsth()

            S_bf = dsl.tile([128, 4 * DV], BF16, tag="sbf", name="sbf")
            nc.vector.tensor_copy(S_bf[:], S_cur[:])
            otA = dp2.tile([128, HP * DV], F32, tag="otA")
            osum8 = dp2.tile([128, HP], F32, tag="osum8")
            s_new = spool.tile([128, 4 * DV], F32, tag="s")

            # breadth-first over groups of 4 heads (2 j-blocks) to keep all
            # engines fed: per stage, 4 independent heads' ops back-to-back
            def kq_ap(S, h):
                j, hh = divmod(h, 2)
                rh = 64 * hh
                return S[rh:rh + 48, 1024 * j + 128 * ci:1024 * j + 128 * ci + 128]

            for grp in range(2):
                js = (2 * grp, 2 * grp + 1)
                hs = [2 * j + hh for j in js for hh in range(2)]
                pA, dte, dincl, wcol, dsm, xx, abar, xt = {}, {}, {}, {}, {}, {}, {}, {}
                for h in hs:
                    pA[h] = cpst()
                    nc.tensor.matmul(pA[h][:, 0:128], kq_ap(kS, h), kq_ap(kS, h),
                                     start=True, stop=True)
                    nc.tensor.matmul(pA[h][:, 128:256], kq_ap(kS, h), kq_ap(qS, h),
                                     start=True, stop=True)
                    nc.tensor.matmul(pA[h][:, 256:384], oneh[:, 128 * h:128 * h + 128],
                                     b_fm[:], start=True, stop=True)
                for h in hs:
                    dte[h] = df32.tile([128, 128], F32, tag="d32", name="dte")
                    nc.vector.scalar_tensor_tensor(
                        dte[h][:], pA[h][:, 256:384], bcum[:, h:h + 1], negl[:],
                        op0=OP.subtract, op1=OP.add)
                for h in hs:
                    dincl[h] = b128()
                    nc.scalar.activation(dincl[h][:], dte[h][:], AF.Exp)
                    wcol[h] = dp3.tile([128, 1], F32, tag="wcol", name="wcol")
                    nc.scalar.activation(wcol[h][:], dte[h][:, 127:128], AF.Exp)
                for h in hs:
                    dsm[h] = b128()
                    nc.gpsimd.tensor_mul(dsm[h][:], dincl[h][:], mst[:])
                for h in hs:
                    xx[h] = b128()
                    nc.vector.scalar_tensor_tensor(
                        xx[h][:], pA[h][:, 0:128], nbeta[:, h:h + 1], dsm[h][:],
                        op0=OP.mult, op1=OP.mult)
                for h in hs:
                    abar[h] = b128()
                    nc.vector.tensor_mul(abar[h][:], pA[h][:, 128:256], dincl[h][:])
                for h in hs:
                    nc.tensor.transpose(pXX[:, 128 * h:128 * h + 128], xx[h][:], idh[:])
                pm, xxa, xta = {}, {}, {}
                for idx, h in enumerate(hs):
                    xt[h] = b128()
                    if idx % 2 == 0:
                        nc.scalar.copy(xt[h][:], pXX[:, 128 * h:128 * h + 128])
                    else:
                        nc.vector.tensor_copy(xt[h][:], pXX[:, 128 * h:128 * h + 128])
                for h in hs:
                    t = b128()
                    nc.gpsimd.tensor_add(t[:], xx[h][:], idh[:])
                    pm[h] = t[:]
                    xxa[h], xta[h] = xx[h][:], xt[h][:]
                # UT doubling, 4 heads interleaved; pU bank: X^2@0, (X^2)^T@128, P@256
                for lvl in range(6):
                    last = lvl == 5
                    pU = {}
                    for h in hs:
                        pU[h] = cpst()
                        if not last:
                            nc.tensor.matmul(pU[h][:, 0:128], xta[h], xxa[h],
                                             start=True, stop=True)
                        nc.tensor.matmul(pU[h][:, 128:256], xxa[h], xta[h],
                                         start=True, stop=True)
                    xtn = {}
                    for idx, h in enumerate(hs):
                        if not last:
                            xp = dxp.tile([128, 256], BF16, tag="xpair", name="xp")
                            if (lvl + idx) % 2 == 0:
                                nc.vector.tensor_copy(xp[:], pU[h][:, 0:256])
                            else:
                                nc.scalar.copy(xp[:], pU[h][:, 0:256])
                            xtn[h] = xp[:, 128:256]
                            xxa[h] = xp[:, 0:128]
                        else:
                            t = b128()
                            nc.vector.tensor_copy(t[:], pU[h][:, 128:256])
                            xtn[h] = t[:]
                    for h in hs:
                        nc.tensor.matmul(pU[h][:, 256:384], idh[:], pm[h],
                                         start=True, stop=False)
                        nc.tensor.matmul(pU[h][:, 256:384], xtn[h], pm[h],
                                         start=False, stop=True)
                    for idx, h in enumerate(hs):
                        t = b128()
                        if (lvl + idx) % 2 == 0:
                            nc.scalar.copy(t[:], pU[h][:, 256:384])
                        else:
                            nc.vector.tensor_copy(t[:], pU[h][:, 256:384])
                        pm[h] = t[:]
                        xta[h] = xtn[h]
                # attention/state matmuls; pV bank: ks@0, w@128, oi@256, qs@384
                p_s = cpst()
                psc = {js[0]: 0, js[1]: 256}
                pV, r_, u_ = {}, {}, {}
                kw = {j: b128() for j in js}
                for h in hs:
                    j, hh = divmod(h, 2)
                    rh = 64 * hh
                    pV[h] = cpst()
                    nc.tensor.matmul(pV[h][:, 0:DV], kq_ap(kS, h),
                                     S_bf[rh:rh + 48, DV * j:DV * j + DV], start=True, stop=True)
                for h in hs:
                    r_[h] = dsl.tile([128, DV], BF16, tag="rr", name="rr")
                    nc.vector.scalar_tensor_tensor(
                        r_[h][:], pV[h][:, 0:DV], nlam[:, h:h + 1], v_tok[:, DV * h:DV * h + DV],
                        op0=OP.mult, op1=OP.add)
                for h in hs:
                    nc.tensor.matmul(pV[h][:, 128:128 + DV], pm[h], r_[h][:],
                                     start=True, stop=True)
                for h in hs:
                    u_[h] = dsl.tile([128, DV], BF16, tag="uu", name="uu")
                    nc.vector.tensor_scalar_mul(u_[h][:], pV[h][:, 128:128 + DV],
                                                beta[:, h:h + 1])
                for h in hs:
                    j, hh = divmod(h, 2)
                    rh = 64 * hh
                    nc.vector.tensor_scalar_mul(
                        kw[j][:, rh:rh + 48], pKt[:, 128 * j + rh:128 * j + rh + 48],
                        wcol[h][:])
                for h in hs:
                    j, hh = divmod(h, 2)
                    rh = 64 * hh
                    nc.tensor.matmul(pV[h][:, 256:256 + DV], abar[h][:], u_[h][:],
                                     start=True, stop=True)
                    nc.tensor.matmul(pV[h][:, 384:384 + DV], kq_ap(qS, h),
                                     S_bf[rh:rh + 48, DV * j:DV * j + DV], start=True, stop=True)
                    nc.tensor.matmul(p_s[rh:rh + 48, psc[j]:psc[j] + DV],
                                     kw[j][:, rh:rh + 48], u_[h][:], start=True, stop=True)
                for h in hs:
                    nc.vector.tensor_scalar_mul(
                        otA[:, DV * h:DV * h + DV], pV[h][:, 384:384 + DV], lam[:, h:h + 1])
                    nc.vector.tensor_add(
                        otA[:, DV * h:DV * h + DV], otA[:, DV * h:DV * h + DV],
                        pV[h][:, 256:256 + DV])
                for h in hs:
                    osq = dp3.tile([128, DV], F32, tag="osq", name="osq")
                    nc.vector.scalar_tensor_tensor(
                        osq[:], otA[:, DV * h:DV * h + DV], 1.0, otA[:, DV * h:DV * h + DV],
                        op0=OP.mult, op1=OP.mult, accum_out=osum8[:, h:h + 1])
                for j in js:
                    for rh2 in (0, 64):
                        nc.vector.scalar_tensor_tensor(
                            s_new[rh2:rh2 + 48, DV * j:DV * j + DV],
                            S_cur[rh2:rh2 + 48, DV * j:DV * j + DV],
                            ebc4[rh2:rh2 + 48, j:j + 1], p_s[rh2:rh2 + 48, psc[j]:psc[j] + DV],
                            op0=OP.mult, op1=OP.add)
            S_cur = s_new

            # per-chunk epilogue: one sqrt for all 8 heads, then gate+transpose to oS
            nc.scalar.activation(osum8[:], osum8[:], AF.Sqrt, bias=epsg[:], scale=1.0 / DV)
            nc.vector.reciprocal(osum8[:], osum8[:])
            for h in range(HP):
                oute = dp3.tile([128, DV], F32, tag="oute")
                nc.vector.scalar_tensor_tensor(
                    oute[:], otA[:, DV * h:DV * h + DV], osum8[:, h:h + 1],
                    g_tok[:, VD_C * ci + DV * h:VD_C * ci + DV * h + DV],
                    op0=OP.mult, op1=OP.mult)
                pOt = cpst()
                nc.tensor.transpose(pOt[0:DV, 0:128], oute[:], idn[:])
                nc.scalar.copy(oS[0:DV, 1024 * h + 128 * ci:1024 * h + 128 * ci + 128],
                               pOt[0:DV, 0:128])

            # interleaved o_proj for this token block (Phase D work)
            if do_d:
                for dh in range(4):
                    pp = cpst()
                    for bb in range(HP):
                        wt = wp2.tile([DV, 512], BF16, tag="wo")
                        nc.sync.dma_start(
                            wt[:], wo_d[DV * bb:DV * bb + DV, 512 * dh:512 * dh + 512])
                        nc.tensor.matmul(
                            pp[:], oS[0:DV, 1024 * bb + 128 * ci:1024 * bb + 128 * ci + 128],
                            wt[:], start=(bb == 0), stop=(bb == HP - 1))
                    stg = pd.tile([128, 512], BF16, tag="s512")
                    nc.scalar.copy(stg[:], pp[:])
                    nc.sync.dma_start(
                        o_in[128 * ci:128 * ci + 128, 512 * dh:512 * dh + 512], stg[:])
                if ci % 2 == 1:
                    q0 = (ci // 2) * 256
                    nc.gpsimd.collective_compute(
                        "AllReduce", OP.add, ins=[o_in[q0:q0 + 256, :]],
                        outs=[o_out[q0:q0 + 256, :]], replica_groups=groups)

        for p in (psCh, psC, pd, wp2, spool, dp3, dp2, df32, dsl, dxp, dbf):
            p.release()
        qkvp.release()
        oSp.release()

        # ============ Phase E ============
        if "D" not in PHASES:
            raise _SkipRest()
        if "E" not in PHASES:
            raise _SkipRest()
        stE = tc.alloc_tile_pool(name="stE", bufs=3)
        psT = tc.alloc_tile_pool(name="psT", bufs=4, space="PSUM")
        ffT = hT
        for i in range(NTOK):
            xa = stE.tile([128, D], F32, tag="x2k")
            nc.sync.dma_start(xa[:], x_d[128 * i:128 * i + 128, :])
            ob = stE.tile([128, D], F32, tag="x2k")
            nc.gpsimd.dma_start(ob[:], o_out[128 * i:128 * i + 128, :])
            nc.vector.tensor_add(xa[:], xa[:], ob[:])
            nc.sync.dma_start(h2_scr[128 * i:128 * i + 128, :], xa[:])
            rcol = stE.tile([128, 1], F32, tag="rcol")
            nc.vector.tensor_mul(ob[:], xa[:], xa[:])
            nc.vector.tensor_reduce(rcol[:], ob[:], mybir.AxisListType.X, OP.add)
            nc.scalar.activation(rcol[:], rcol[:], AF.Sqrt, bias=eps1[:], scale=1.0 / D)
            nc.vector.reciprocal(rcol[:], rcol[:])
            xb = stE.tile([128, D], BF16, tag="xb")
            nc.vector.tensor_scalar_mul(xb[:], xa[:], rcol[:])
            for k in range(KT):
                pt = psT.tile([128, 128], BF16, tag="pt", name="ptE")
                nc.tensor.transpose(pt[:], xb[:, 128 * k:128 * k + 128], idh[:])
                if k % 2 == 0:
                    nc.scalar.copy(ffT[:, 1024 * k + 128 * i:1024 * k + 128 * i + 128], pt[:])
                else:
                    nc.vector.tensor_copy(ffT[:, 1024 * k + 128 * i:1024 * k + 128 * i + 128], pt[:])
        psT.release()
        psDE = tc.alloc_tile_pool(name="psDE", bufs=8, space="PSUM")
        pdE = tc.alloc_tile_pool(name="pdE", bufs=4)
        wpE = tc.alloc_tile_pool(name="wpE", bufs=7)

        def pst2(p=128, f=512):
            return psDE.tile([p, f], F32, tag="ps2", name="pst2")

        pgE = tc.alloc_tile_pool(name="pgE", bufs=1)
        mida = pgE.tile([128, 6 * 1024], BF16, tag="mida")
        pmid = tc.alloc_tile_pool(name="pmid", bufs=1)
        midb = pmid.tile([128, 5 * 1024], BF16, tag="midb")

        def mid_ap(m, off, ln):
            if m < 6:
                return mida[:, 1024 * m + off:1024 * m + off + ln]
            return midb[:, 1024 * (m - 6) + off:1024 * (m - 6) + off + ln]

        for mb in range(0, 12, 2):
            ms = [m for m in (mb, mb + 1) if m < 11]
            if not ms:
                break
            wid = 128 * len(ms)
            pus = {m: [pst2() for _ in range(2)] for m in ms}
            pvs = {m: [pst2() for _ in range(2)] for m in ms}
            for k in range(KT):
                wt1 = wp.tile([128, 256], BF16, tag="w")
                nc.sync.dma_start(
                    wt1[:, 0:wid],
                    w1_d[128 * k:128 * k + 128, 128 * mb:128 * mb + wid])
                wt3 = wp.tile([128, 256], BF16, tag="w")
                nc.sync.dma_start(
                    wt3[:, 0:wid],
                    w3_d[128 * k:128 * k + 128, 128 * mb:128 * mb + wid])
                for mi, m in enumerate(ms):
                    for n in range(2):
                        rhs = ffT[:, 1024 * k + 512 * n:1024 * k + 512 * n + 512]
                        nc.tensor.matmul(pus[m][n][:], wt1[:, 128 * mi:128 * mi + 128], rhs,
                                         start=(k == 0), stop=(k == KT - 1))
                        nc.tensor.matmul(pvs[m][n][:], wt3[:, 128 * mi:128 * mi + 128], rhs,
                                         start=(k == 0), stop=(k == KT - 1))
            for m in ms:
                for n in range(2):
                    u1s = pdE.tile([128, 512], F32, tag="s512")
                    nc.scalar.activation(u1s[:], pus[m][n][:], AF.Silu)
                    nc.vector.tensor_mul(mid_ap(m, 512 * n, 512), u1s[:], pvs[m][n][:])

        for dh in range(4):
            pps = [pst2() for _ in range(NTOK)]
            for mgrp in (range(0, 6), range(6, 11)):
                for m in mgrp:
                    wt = wpE.tile([128, 512], BF16, tag="w512")
                    nc.sync.dma_start(
                        wt[:], w2_d[128 * m:128 * m + 128, 512 * dh:512 * dh + 512])
                    for i in range(NTOK):
                        nc.tensor.matmul(pps[i][:], mid_ap(m, 128 * i, 128), wt[:],
                                         start=(m == 0), stop=(m == 10))
            for i in range(NTOK):
                h2t = pdE.tile([128, 512], F32, tag="s512")
                nc.sync.dma_start(h2t[:], h2_scr[128 * i:128 * i + 128, 512 * dh:512 * dh + 512])
                yst = pdE.tile([128, 512], F32, tag="s512")
                nc.vector.tensor_scalar_mul(yst[:], h2t[:], 0.25)
                nc.vector.tensor_add(yst[:], yst[:], pps[i][:])
                nc.sync.dma_start(y_d[128 * i:128 * i + 128, 512 * dh:512 * dh + 512], yst[:])

        for p in (pmid, pgE, wpE, pdE, stE, psDE, dram, wp, pg, big, cpool):
            p.release()
      except _SkipRest:
        zst = tc.alloc_tile_pool(name="zst", bufs=1)
        zt = zst.tile([128, 512], F32)
        nc.vector.memset(zt[:], 0.0)
        for i in range(NTOK):
            for dh in range(4):
                nc.sync.dma_start(y_d[128 * i:128 * i + 128, 512 * dh:512 * dh + 512], zt[:])
        zst.release()
        for pname in ("psCh", "psC", "pd", "wp2", "spool", "dp3", "dp2", "df32", "dsl",
                      "dxp", "dbf", "qkvp", "oSp", "pb", "psAB", "psDE", "stE", "dram", "wp",
                      "pg", "big", "cpool"):
            p = locals().get(pname)
            if p is not None:
                try:
                    p.release()
                except Exception:
                    pass

    nc.compile()
    return nc


def _shard(inputs):
    import ml_dtypes
    bf16 = ml_dtypes.bfloat16
    f32 = np.float32
    rms1 = np.asarray(inputs["rms1_w"], f32)
    rms2 = np.asarray(inputs["rms2_w"], f32)
    gn = np.asarray(inputs["gnorm_w"], f32)
    in_maps = []
    for c in range(8):
        g, m = c // 4, c % 4
        qs = slice(384 * m, 384 * m + 384)
        vs = slice(768 * m, 768 * m + 768)
        hs = slice(8 * m, 8 * m + 8)
        isl = slice(1408 * m, 1408 * m + 1408)

        def padqk(w):
            wp_ = np.zeros((D, QKP), f32)
            for h in range(8):
                wp_[:, 64 * h:64 * h + 48] = w[:, 48 * h:48 * h + 48]
            return wp_

        def padcw(w):
            cp = np.zeros((QKP, 4), f32)
            for h in range(8):
                cp[64 * h:64 * h + 48] = w[48 * h:48 * h + 48]
            return cp

        def padv(w):
            colpad = w.shape[0] == D
            out = np.zeros((D, VP) if colpad else (VP, w.shape[1]), f32)
            for h in range(8):
                if colpad:
                    out[:, 128 * h:128 * h + 96] = w[:, 96 * h:96 * h + 96]
                else:
                    out[128 * h:128 * h + 96] = w[96 * h:96 * h + 96]
            return out

        in_maps.append(dict(
            x=np.ascontiguousarray(np.asarray(inputs["x"], f32)[g]),
            wq=padqk(np.asarray(inputs["Wq"], f32)[:, qs] * rms1[:, None]).astype(bf16),
            wk=padqk(np.asarray(inputs["Wk"], f32)[:, qs] * rms1[:, None]).astype(bf16),
            wv=padv(np.asarray(inputs["Wv"], f32)[:, vs] * rms1[:, None]).astype(bf16),
            wg=np.ascontiguousarray(
                np.asarray(inputs["Wg"], f32)[:, vs] * rms1[:, None]).astype(bf16),
            wab=np.ascontiguousarray(np.concatenate(
                [np.asarray(inputs["Wa"], f32)[:, hs],
                 np.asarray(inputs["Wb"], f32)[:, hs]], 1) * rms1[:, None]),
            cq=padcw(np.asarray(inputs["conv_q_w"], f32)[qs]),
            ck=padcw(np.asarray(inputs["conv_k_w"], f32)[qs]),
            cv=padv(np.asarray(inputs["conv_v_w"], f32)[vs]),
            dtb=np.asarray(inputs["dt_bias"], f32)[hs].reshape(1, 8).copy(),
            nega=(-np.exp(np.asarray(inputs["A_log"], f32)[hs])).reshape(1, 8).copy(),
            wo=np.ascontiguousarray(
                np.asarray(inputs["Wo"], f32)[vs] * np.tile(gn, 8)[:, None]).astype(bf16),
            w1=np.ascontiguousarray(
                np.asarray(inputs["W1"], f32)[:, isl] * rms2[:, None]).astype(bf16),
            w3=np.ascontiguousarray(
                np.asarray(inputs["W3"], f32)[:, isl] * rms2[:, None]).astype(bf16),
            w2=np.ascontiguousarray(np.asarray(inputs["W2"], f32)[isl]).astype(bf16),
        ))
    return in_maps


def kernel(**inputs):
    if "nc" not in _cache:
        _cache["nc"] = _build(8)
    res = run_bass_kernel_spmd(_cache["nc"], _shard(inputs), list(range(8)))
    out = np.zeros((B, T, D), np.float32)
    for g in range(2):
        out[g] = sum(res.results[4 * g + m]["y"] for m in range(4))
    return out


# revision 70
# speedup vs baseline: 1.0488x; 1.0488x over previous
"""GatedDeltaNet block kernel for 8 Trainium2 cores (Bass/Tile).

Sharding: DP2 (batch) x TP4 (heads / MLP-inter). Core c: group g=c//4 runs
batch g; member m=c%4 owns heads [8m,8m+8), q/k cols [384m,..), v/g cols
[768m,..), INTER [1408m,..). One on-device AllReduce per 4-core group after
o_proj; final down-proj partials summed on the host.

Per-core dataflow:
  A: x -> rmsnorm (token-major) -> PE-transpose -> hT [2048,1024] f32r (SBUF)
  B: fp32r projections off hT; q/k feature-major (heads padded to 64 rows)
     -> conv+silu+l2norm -> bf16 SBUF qS/kS; v -> conv+silu -> bf16 SBUF vS;
     gate token-major direct (silu at evict) -> SBUF; a/b -> SBUF
  C: chunked gated delta rule (C=128, UT transform via log-doubling inverse)
     with bf16 matmul operands / fp32 psum+state; fused DVE ops; writes
     normed+gated o to DRAM (f32r)
  D: o_proj token-major partial -> AllReduce (group of 4)
  E: h2 = x + o_sum; rmsnorm -> transpose -> ffT (reuses hT slot); MLP f32r;
     y = W2_partial + 0.25*h2  (host sums the 4 partials per group)
"""
import sys
sys.path.insert(0, '/opt/trn_rl_repo')
import numpy as np

import concourse.bass as bass
import concourse.bacc as bacc
import concourse.mybir as mybir
import concourse.tile as tile
from concourse.bass_isa import ReduceOp
from concourse.bass_utils import run_bass_kernel_spmd

F32 = mybir.dt.float32
F32R = mybir.dt.float32r
BF16 = mybir.dt.bfloat16
AF = mybir.ActivationFunctionType
OP = mybir.AluOpType

B, T, D = 2, 1024, 2048
H, DK, DV = 32, 48, 96
HP = 8
QKP = 512
VD_C = 768
VP = 1024
INT_C = 1408
C = 128
NCHUNK = T // C
KT = D // 128
NTOK = T // 128

_cache = {}
import os
PHASES = os.environ.get("DN_PHASES", "ABCDE")
NCH = int(os.environ.get("DN_NCHUNK", str(T // C)))


class _SkipRest(Exception):
    pass


def _build(n_cores=8):
    groups = [[0, 1, 2, 3], [4, 5, 6, 7]] if n_cores == 8 else [[0]]
    nc = bacc.Bacc("TRN2", target_bir_lowering=False, debug=False, num_devices=n_cores)

    x_d = nc.dram_tensor("x", [T, D], F32, kind="ExternalInput")
    wq_d = nc.dram_tensor("wq", [D, QKP], BF16, kind="ExternalInput")
    wk_d = nc.dram_tensor("wk", [D, QKP], BF16, kind="ExternalInput")
    wv_d = nc.dram_tensor("wv", [D, VP], BF16, kind="ExternalInput")
    wg_d = nc.dram_tensor("wg", [D, VD_C], BF16, kind="ExternalInput")
    wab_d = nc.dram_tensor("wab", [D, 16], F32, kind="ExternalInput")
    cq_d = nc.dram_tensor("cq", [QKP, 4], F32, kind="ExternalInput")
    ck_d = nc.dram_tensor("ck", [QKP, 4], F32, kind="ExternalInput")
    cv_d = nc.dram_tensor("cv", [VP, 4], F32, kind="ExternalInput")
    dtb_d = nc.dram_tensor("dtb", [1, HP], F32, kind="ExternalInput")
    nega_d = nc.dram_tensor("nega", [1, HP], F32, kind="ExternalInput")
    wo_d = nc.dram_tensor("wo", [VD_C, D], BF16, kind="ExternalInput")
    w1_d = nc.dram_tensor("w1", [D, INT_C], BF16, kind="ExternalInput")
    w3_d = nc.dram_tensor("w3", [D, INT_C], BF16, kind="ExternalInput")
    w2_d = nc.dram_tensor("w2", [INT_C, D], BF16, kind="ExternalInput")
    y_d = nc.dram_tensor("y", [T, D], F32, kind="ExternalOutput")

    idn_c = nc.inline_tensor(np.eye(128, dtype=np.float32), "idn_c")
    ones = np.ones((128, 128), np.float32)
    cum_c = nc.inline_tensor(np.triu(ones).copy(), "cum_c")
    mst_c = nc.inline_tensor(np.triu(ones, 1).copy(), "mst_c")
    negl_c = nc.inline_tensor((np.tril(ones, -1) * -1e30).copy(), "negl_c")
    sel_np = np.zeros((HP, 512), np.float32)
    for j in range(4):
        sel_np[2 * j, 128 * j:128 * j + 48] = 1.0
        sel_np[2 * j + 1, 128 * j + 64:128 * j + 112] = 1.0
    sel_c = nc.inline_tensor(sel_np, "sel_c")
    on48_np = np.zeros((128, 2), np.float32)
    on48_np[0:48, 0] = 1.0
    on48_np[64:112, 1] = 1.0
    on48_c = nc.inline_tensor(on48_np, "on48_c")
    oneh_np = np.zeros((HP, HP * 128), np.float32)
    for h in range(HP):
        oneh_np[h, 128 * h:128 * h + 128] = 1.0
    oneh_c = nc.inline_tensor(oneh_np, "oneh_c")

    with tile.TileContext(nc) as tc:
      try:
        cpool = tc.alloc_tile_pool(name="consts", bufs=1)
        big = tc.alloc_tile_pool(name="big", bufs=1)
        pg = tc.alloc_tile_pool(name="pg", bufs=1)
        wp = tc.alloc_tile_pool(name="wp", bufs=6)
        oSp = tc.alloc_tile_pool(name="oSp", bufs=1)
        qkvp = tc.alloc_tile_pool(name="qkvp", bufs=1)
        dram = tc.alloc_tile_pool(name="dram", bufs=1, space="DRAM")

        idn = cpool.tile([128, 128], F32)
        idh = cpool.tile([128, 128], BF16)
        cum = cpool.tile([128, 128], F32)
        mst = cpool.tile([128, 128], BF16)
        negl = cpool.tile([128, 128], F32)
        sel = cpool.tile([HP, 512], F32)
        on48 = cpool.tile([128, 2], F32)
        oneh = cpool.tile([HP, HP * 128], F32)
        for t_, s_ in [(idn, idn_c), (cum, cum_c), (sel, sel_c),
                       (negl, negl_c), (oneh, oneh_c)]:
            nc.sync.dma_start(t_[:], s_[:])
        nc.gpsimd.dma_start(mst[:], mst_c[:])
        nc.sync.dma_start(on48[:], on48_c[:])
        nc.vector.tensor_copy(idh[:], idn[:])
        eps1 = cpool.tile([128, 1], F32)
        nc.vector.memset(eps1[:], 1e-5)
        epsq = cpool.tile([128, 1], F32)
        nc.vector.memset(epsq[:], 48e-6)
        epsk = cpool.tile([128, 1], F32)
        nc.vector.memset(epsk[:], 1e-6)
        epsg = cpool.tile([128, 1], F32)
        nc.vector.memset(epsg[:], 1e-5)
        dtb_r = cpool.tile([1, HP], F32)
        nega_r = cpool.tile([1, HP], F32)
        nc.sync.dma_start(dtb_r[:], dtb_d[:])
        nc.sync.dma_start(nega_r[:], nega_d[:])
        dtb_bc = cpool.tile([128, HP], F32)
        nega_bc = cpool.tile([128, HP], F32)
        nc.gpsimd.partition_broadcast(dtb_bc[:], dtb_r[:])
        nc.gpsimd.partition_broadcast(nega_bc[:], nega_r[:])
        cqw = cpool.tile([128, 16], F32)
        ckw = cpool.tile([128, 16], F32)
        cvw = cpool.tile([128, 32], F32)
        for j in range(4):
            nc.sync.dma_start(cqw[:, 4 * j:4 * j + 4], cq_d[128 * j:128 * j + 128, :])
            nc.sync.dma_start(ckw[:, 4 * j:4 * j + 4], ck_d[128 * j:128 * j + 128, :])
        for j in range(8):
            nc.sync.dma_start(cvw[:, 4 * j:4 * j + 4], cv_d[128 * j:128 * j + 128, :])
        wab_s = cpool.tile([128, 16 * KT], F32)
        for k in range(KT):
            nc.sync.dma_start(wab_s[:, 16 * k:16 * k + 16], wab_d[128 * k:128 * k + 128, :])
        ab_fm = cpool.tile([16, 1024], F32)

        hT = big.tile([128, KT * 1024], BF16)
        g_tok = pg.tile([128, NTOK * VD_C], BF16, tag="gtok")
        # o kept SBUF-resident, per-head 128-col token blocks (rows 0:96 used)
        oS = oSp.tile([128, HP * 1024], BF16, tag="oS")
        qS = qkvp.tile([128, 4 * 1024], BF16, tag="qS")
        kS = qkvp.tile([128, 4 * 1024], BF16, tag="kS")
        vS = qkvp.tile([128, 8 * 1024], BF16, tag="vS")
        nc.vector.memset(qS[:], 0.0)
        nc.vector.memset(kS[:], 0.0)

        o_in = dram.tile([T, D], BF16)
        o_out = dram.tile([T, D], BF16)
        h2_scr = dram.tile([T, D], F32)

        # ============ Phase A ============
        psAB = tc.alloc_tile_pool(name="psAB", bufs=8, space="PSUM")

        def pst(p=128, f=512):
            return psAB.tile([p, f], F32, tag="ps", name="pst")

        stA = tc.alloc_tile_pool(name="stA", bufs=3)
        stA2 = tc.alloc_tile_pool(name="stA2", bufs=4)
        for i in range(NTOK):
            xa = stA.tile([128, D], F32, tag="x2k")
            nc.sync.dma_start(xa[:], x_d[128 * i:128 * i + 128, :])
            sq = stA.tile([128, D], F32, tag="x2k")
            rcol = stA.tile([128, 1], F32, tag="rcol")
            nc.vector.tensor_mul(sq[:], xa[:], xa[:])
            nc.vector.tensor_reduce(rcol[:], sq[:], mybir.AxisListType.X, OP.add)
            nc.scalar.activation(rcol[:], rcol[:], AF.Sqrt, bias=eps1[:], scale=1.0 / D)
            nc.vector.reciprocal(rcol[:], rcol[:])
            nc.vector.tensor_scalar_mul(xa[:], xa[:], rcol[:])
            p_abi = pst(16, 128)
            for k in range(KT):
                pt = pst(128, 128)
                nc.tensor.transpose(pt[:], xa[:, 128 * k:128 * k + 128], idn[:])
                st32 = stA2.tile([128, 128], F32, tag="st32")
                if k % 2 == 0:
                    nc.scalar.copy(st32[:], pt[:])
                else:
                    nc.vector.tensor_copy(st32[:], pt[:])
                nc.gpsimd.tensor_copy(hT[:, 1024 * k + 128 * i:1024 * k + 128 * i + 128], st32[:])
                nc.tensor.matmul(p_abi[:], wab_s[:, 16 * k:16 * k + 16], st32[:],
                                 start=(k == 0), stop=(k == KT - 1))
            nc.vector.tensor_copy(ab_fm[:, 128 * i:128 * i + 128], p_abi[:])
        stA2.release()
        stA.release()

        # ============ Phase B ============
        if "B" not in PHASES:
            raise _SkipRest()
        pb = tc.alloc_tile_pool(name="pb", bufs=6)

        def conv_silu(pre, cw, j, out_ap):
            acc = pb.tile([128, 1024], F32, tag="s1k")
            nc.scalar.activation(acc[:], pre[:], AF.Copy, scale=cw[:, 4 * j + 3:4 * j + 4])
            for s in (1, 2, 3):
                nc.vector.scalar_tensor_tensor(
                    acc[:, s:1024], pre[:, 0:1024 - s], cw[:, 4 * j + 3 - s:4 * j + 4 - s],
                    acc[:, s:1024], op0=OP.mult, op1=OP.add)
            nc.scalar.activation(out_ap, acc[:], AF.Silu)

        def qkv_pass(w_dram, outS, cw, eps_col, mult, do_l2, jbase, wcol0):
            # one pass: 4 feature blocks, k-outer, 8 psums, 1 wide DMA per k
            pps = [[pst() for n in range(2)] for j in range(4)]
            for k in range(KT):
                wt = wp.tile([128, 512], BF16, tag="wwide")
                nc.sync.dma_start(
                    wt[:], w_dram[128 * k:128 * k + 128, wcol0:wcol0 + 512])
                for j in range(4):
                    for n in range(2):
                        nc.tensor.matmul(
                            pps[j][n][:], wt[:, 128 * j:128 * j + 128],
                            hT[:, 1024 * k + 512 * n:1024 * k + 512 * n + 512],
                            start=(k == 0), stop=(k == KT - 1))
            for j in range(4):
                jj = jbase + j
                pre = pb.tile([128, 1024], F32, tag="s1k")
                for n in range(2):
                    nc.vector.tensor_copy(pre[:, 512 * n:512 * n + 512], pps[j][n][:])
                if not do_l2:
                    conv_silu(pre, cw, jj, outS[:, 1024 * jj:1024 * jj + 1024])
                    continue
                blk = pb.tile([128, 1024], F32, tag="s1k")
                conv_silu(pre, cw, jj, blk[:])
                sq = pb.tile([128, 1024], F32, tag="s1k")
                nc.vector.tensor_mul(sq[:], blk[:], blk[:])
                for hh, rh in ((0, 0), (1, 64)):
                    srow = pb.tile([1, 1024], F32, tag="srow")
                    for n2 in range(2):
                        p_ssq = pst(1, 512)
                        nc.tensor.matmul(
                            p_ssq[:], on48[:, hh:hh + 1], sq[:, 512 * n2:512 * n2 + 512],
                            start=True, stop=True)
                        nc.scalar.activation(srow[:, 512 * n2:512 * n2 + 512], p_ssq[:],
                                             AF.Sqrt, bias=eps_col[0:1, :], scale=mult)
                    sbc = pb.tile([128, 1024], F32, tag="s1k")
                    nc.gpsimd.partition_broadcast(sbc[:], srow[:])
                    nc.vector.reciprocal(sbc[rh:rh + 48, :], sbc[rh:rh + 48, :])
                    nc.vector.tensor_mul(
                        outS[rh:rh + 48, 1024 * jj:1024 * jj + 1024],
                        blk[rh:rh + 48, :], sbc[rh:rh + 48, :])

        qkv_pass(wq_d, qS, cqw, epsq, 48.0, True, 0, 0)
        qkv_pass(wk_d, kS, ckw, epsk, 1.0, True, 0, 0)
        qkv_pass(wv_d, vS, cvw, None, None, False, 0, 0)
        qkv_pass(wv_d, vS, cvw, None, None, False, 4, 512)

        # gate token-major
        for n in range(2):
            pgs = [pst(128, 384) for _ in range(NTOK)]
            for k in range(KT):
                wt = wp.tile([128, 384], BF16, tag="wg384")
                nc.sync.dma_start(
                    wt[:], wg_d[128 * k:128 * k + 128, 384 * n:384 * n + 384])
                for i in range(NTOK):
                    nc.tensor.matmul(
                        pgs[i][:], hT[:, 1024 * k + 128 * i:1024 * k + 128 * i + 128], wt[:],
                        start=(k == 0), stop=(k == KT - 1))
            for i in range(NTOK):
                nc.scalar.activation(
                    g_tok[:, VD_C * i + 384 * n:VD_C * i + 384 * n + 384], pgs[i][:], AF.Silu)
        pb.release()
        psAB.release()

        # ============ Phase C (+ interleaved Phase D o_proj) ============
        if "C" not in PHASES:
            raise _SkipRest()
        dbf = tc.alloc_tile_pool(name="dbf", bufs=26)
        dxp = tc.alloc_tile_pool(name="dxp", bufs=10)
        dsl = tc.alloc_tile_pool(name="dsl", bufs=6)
        df32 = tc.alloc_tile_pool(name="df32", bufs=5)
        dp2 = tc.alloc_tile_pool(name="dp2", bufs=2)
        dp3 = tc.alloc_tile_pool(name="dp3", bufs=6)
        spool = tc.alloc_tile_pool(name="spool", bufs=2)
        wp2 = tc.alloc_tile_pool(name="wp2", bufs=9)
        pd = tc.alloc_tile_pool(name="pd", bufs=3)
        psC = tc.alloc_tile_pool(name="psC", bufs=5, space="PSUM")
        psCh = tc.alloc_tile_pool(name="psCh", bufs=3, space="PSUM")

        def cpst():
            return psC.tile([128, 512], F32, tag="c", name="cpst")

        def cpsth():
            return psCh.tile([128, 1024], BF16, tag="ch", name="cpsth")

        def b128():
            return dbf.tile([128, 128], BF16, tag="b128", name="b128")

        S_cur = spool.tile([128, 4 * DV], F32, tag="s", name="s")
        nc.vector.memset(S_cur[:], 0.0)
        do_d = "D" in PHASES

        for ci in range(NCH):
            cs = slice(128 * ci, 128 * ci + 128)
            # --- per-chunk decay/beta prep (f32); pPrep bank: ab@0, bcum@128, bT@256, ebc4@384
            pPrep = cpst()
            nc.tensor.transpose(pPrep[:, 0:16], ab_fm[:, cs], idn[0:16, 0:16])
            gt = dp2.tile([128, HP], F32, tag="gt")
            nc.vector.tensor_add(gt[:], pPrep[:, 0:HP], dtb_bc[:])
            nc.scalar.activation(gt[:], gt[:], AF.Exp)
            nc.vector.tensor_scalar_add(gt[:], gt[:], 1.0)
            nc.scalar.activation(gt[:], gt[:], AF.Ln)
            nc.vector.tensor_mul(gt[:], gt[:], nega_bc[:])
            beta = dp2.tile([128, HP], F32, tag="beta")
            nc.scalar.activation(beta[:], pPrep[:, HP:16], AF.Sigmoid)
            nbeta = dp2.tile([128, HP], F32, tag="nbeta")
            nc.vector.tensor_scalar_mul(nbeta[:], beta[:], -1.0)
            nc.tensor.matmul(pPrep[:, 128:128 + HP], cum[:], gt[:], start=True, stop=True)
            bcum = dp2.tile([128, HP], F32, tag="bcum")
            nc.vector.tensor_copy(bcum[:], pPrep[:, 128:128 + HP])
            lam = dp2.tile([128, HP], F32, tag="lam")
            nc.scalar.activation(lam[:], pPrep[:, 128:128 + HP], AF.Exp)
            nlam = dp2.tile([128, HP], F32, tag="nlam")
            nc.vector.tensor_scalar_mul(nlam[:], lam[:], -1.0)
            nc.tensor.transpose(pPrep[0:HP, 256:384], bcum[:], idn[:])
            b_fm = dp2.tile([HP, 128], F32, tag="bfm")
            nc.vector.tensor_copy(b_fm[:], pPrep[0:HP, 256:384])
            ebc = dp2.tile([HP, 1], F32, tag="ebc")
            nc.scalar.activation(ebc[:], b_fm[:, 127:128], AF.Exp)
            for j in range(4):
                nc.tensor.matmul(pPrep[:, 384 + j:385 + j], sel[:, 128 * j:128 * j + 128],
                                 ebc[:], start=True, stop=True)
            ebc4 = dp2.tile([128, 4], F32, tag="ebc4")
            nc.vector.tensor_copy(ebc4[:], pPrep[:, 384:388])

            # v token-major: pack all 8 heads' transposes in one bf16 bank
            pVt = cpsth()
            for h in range(HP):
                nc.tensor.transpose(pVt[:, DV * h:DV * h + DV],
                                    vS[0:DV, 1024 * h + 128 * ci:1024 * h + 128 * ci + 128],
                                    idh[0:DV, 0:DV])
            v_tok = dp2.tile([128, HP * DV], F32, tag="vtok")
            nc.vector.tensor_copy(v_tok[:], pVt[:, 0:HP * DV])

            # k token-major (for kw), packed
            pKt = cpsth()
            for j in range(4):
                nc.tensor.transpose(pKt[:, 128 * j:128 * j + 128],
                                    kS[:, 1024 * j + 128 * ci:1024 * j + 128 * ci + 128], idh[:])
            pXX = cpsth()

            S_bf = dsl.tile([128, 4 * DV], BF16, tag="sbf", name="sbf")
            nc.vector.tensor_copy(S_bf[:], S_cur[:])
            otA = dp2.tile([128, HP * DV], F32, tag="otA")
            osum8 = dp2.tile([128, HP], F32, tag="osum8")
            s_new = spool.tile([128, 4 * DV], F32, tag="s")

            # breadth-first over groups of 4 heads (2 j-blocks) to keep all
            # engines fed: per stage, 4 independent heads' ops back-to-back
            def kq_ap(S, h):
                j, hh = divmod(h, 2)
                rh = 64 * hh
                return S[rh:rh + 48, 1024 * j + 128 * ci:1024 * j + 128 * ci + 128]

            for grp in range(2):
                js = (2 * grp, 2 * grp + 1)
                hs = [2 * j + hh for j in js for hh in range(2)]
                pA, dte, dincl, wcol, dsm, xx, abar, xt = {}, {}, {}, {}, {}, {}, {}, {}
                for h in hs:
                    pA[h] = cpst()
                    nc.tensor.matmul(pA[h][:, 0:128], kq_ap(kS, h), kq_ap(kS, h),
                                     start=True, stop=True)
                    nc.tensor.matmul(pA[h][:, 128:256], kq_ap(kS, h), kq_ap(qS, h),
                                     start=True, stop=True)
                    nc.tensor.matmul(pA[h][:, 256:384], oneh[:, 128 * h:128 * h + 128],
                                     b_fm[:], start=True, stop=True)
                for h in hs:
                    dte[h] = df32.tile([128, 128], F32, tag="d32", name="dte")
                    nc.vector.scalar_tensor_tensor(
                        dte[h][:], pA[h][:, 256:384], bcum[:, h:h + 1], negl[:],
                        op0=OP.subtract, op1=OP.add)
                for h in hs:
                    dincl[h] = b128()
                    nc.scalar.activation(dincl[h][:], dte[h][:], AF.Exp)
                    wcol[h] = dp3.tile([128, 1], F32, tag="wcol", name="wcol")
                    nc.scalar.activation(wcol[h][:], dte[h][:, 127:128], AF.Exp)
                for h in hs:
                    dsm[h] = b128()
                    nc.gpsimd.tensor_mul(dsm[h][:], dincl[h][:], mst[:])
                for h in hs:
                    xx[h] = b128()
                    nc.vector.scalar_tensor_tensor(
                        xx[h][:], pA[h][:, 0:128], nbeta[:, h:h + 1], dsm[h][:],
                        op0=OP.mult, op1=OP.mult)
                for h in hs:
                    abar[h] = b128()
                    nc.vector.tensor_mul(abar[h][:], pA[h][:, 128:256], dincl[h][:])
                for h in hs:
                    nc.tensor.transpose(pXX[:, 128 * h:128 * h + 128], xx[h][:], idh[:])
                pm, xxa, xta = {}, {}, {}
                for idx, h in enumerate(hs):
                    xt[h] = b128()
                    if idx % 2 == 0:
                        nc.scalar.copy(xt[h][:], pXX[:, 128 * h:128 * h + 128])
                    else:
                        nc.vector.tensor_copy(xt[h][:], pXX[:, 128 * h:128 * h + 128])
                for h in hs:
                    t = b128()
                    nc.gpsimd.tensor_add(t[:], xx[h][:], idh[:])
                    pm[h] = t[:]
                    xxa[h], xta[h] = xx[h][:], xt[h][:]
                # UT doubling, 4 heads interleaved; pU bank: X^2@0, (X^2)^T@128, P@256
                for lvl in range(6):
                    last = lvl == 5
                    pU = {}
                    for h in hs:
                        pU[h] = cpst()
                        if not last:
                            nc.tensor.matmul(pU[h][:, 0:128], xta[h], xxa[h],
                                             start=True, stop=True)
                        nc.tensor.matmul(pU[h][:, 128:256], xxa[h], xta[h],
                                         start=True, stop=True)
                    xtn = {}
                    for idx, h in enumerate(hs):
                        if not last:
                            xp = dxp.tile([128, 256], BF16, tag="xpair", name="xp")
                            if (lvl + idx) % 2 == 0:
                                nc.vector.tensor_copy(xp[:], pU[h][:, 0:256])
                            else:
                                nc.scalar.copy(xp[:], pU[h][:, 0:256])
                            xtn[h] = xp[:, 128:256]
                            xxa[h] = xp[:, 0:128]
                        else:
                            t = b128()
                            nc.vector.tensor_copy(t[:], pU[h][:, 128:256])
                            xtn[h] = t[:]
                    for h in hs:
                        nc.tensor.matmul(pU[h][:, 256:384], idh[:], pm[h],
                                         start=True, stop=False)
                        nc.tensor.matmul(pU[h][:, 256:384], xtn[h], pm[h],
                                         start=False, stop=True)
                    for idx, h in enumerate(hs):
                        t = b128()
                        if (lvl + idx) % 2 == 0:
                            nc.scalar.copy(t[:], pU[h][:, 256:384])
                        else:
                            nc.vector.tensor_copy(t[:], pU[h][:, 256:384])
                        pm[h] = t[:]
                        xta[h] = xtn[h]
                # attention/state matmuls; pV bank: ks@0, w@128, oi@256, qs@384
                p_s = cpst()
                psc = {js[0]: 0, js[1]: 256}
                pV, r_, u_ = {}, {}, {}
                kw = {j: b128() for j in js}
                for h in hs:
                    j, hh = divmod(h, 2)
                    rh = 64 * hh
                    pV[h] = cpst()
                    nc.tensor.matmul(pV[h][:, 0:DV], kq_ap(kS, h),
                                     S_bf[rh:rh + 48, DV * j:DV * j + DV], start=True, stop=True)
                for h in hs:
                    r_[h] = dsl.tile([128, DV], BF16, tag="rr", name="rr")
                    nc.vector.scalar_tensor_tensor(
                        r_[h][:], pV[h][:, 0:DV], nlam[:, h:h + 1], v_tok[:, DV * h:DV * h + DV],
                        op0=OP.mult, op1=OP.add)
                for h in hs:
                    nc.tensor.matmul(pV[h][:, 128:128 + DV], pm[h], r_[h][:],
                                     start=True, stop=True)
                for h in hs:
                    u_[h] = dsl.tile([128, DV], BF16, tag="uu", name="uu")
                    nc.vector.tensor_scalar_mul(u_[h][:], pV[h][:, 128:128 + DV],
                                                beta[:, h:h + 1])
                for h in hs:
                    j, hh = divmod(h, 2)
                    rh = 64 * hh
                    nc.vector.tensor_scalar_mul(
                        kw[j][:, rh:rh + 48], pKt[:, 128 * j + rh:128 * j + rh + 48],
                        wcol[h][:])
                for h in hs:
                    j, hh = divmod(h, 2)
                    rh = 64 * hh
                    nc.tensor.matmul(pV[h][:, 256:256 + DV], abar[h][:], u_[h][:],
                                     start=True, stop=True)
                    nc.tensor.matmul(pV[h][:, 384:384 + DV], kq_ap(qS, h),
                                     S_bf[rh:rh + 48, DV * j:DV * j + DV], start=True, stop=True)
                    nc.tensor.matmul(p_s[rh:rh + 48, psc[j]:psc[j] + DV],
                                     kw[j][:, rh:rh + 48], u_[h][:], start=True, stop=True)
                for h in hs:
                    nc.vector.tensor_scalar_mul(
                        otA[:, DV * h:DV * h + DV], pV[h][:, 384:384 + DV], lam[:, h:h + 1])
                    nc.vector.tensor_add(
                        otA[:, DV * h:DV * h + DV], otA[:, DV * h:DV * h + DV],
                        pV[h][:, 256:256 + DV])
                for h in hs:
                    osq = dp3.tile([128, DV], F32, tag="osq", name="osq")
                    nc.vector.scalar_tensor_tensor(
                        osq[:], otA[:, DV * h:DV * h + DV], 1.0, otA[:, DV * h:DV * h + DV],
                        op0=OP.mult, op1=OP.mult, accum_out=osum8[:, h:h + 1])
                for j in js:
                    for rh2 in (0, 64):
                        nc.vector.scalar_tensor_tensor(
                            s_new[rh2:rh2 + 48, DV * j:DV * j + DV],
                            S_cur[rh2:rh2 + 48, DV * j:DV * j + DV],
                            ebc4[rh2:rh2 + 48, j:j + 1], p_s[rh2:rh2 + 48, psc[j]:psc[j] + DV],
                            op0=OP.mult, op1=OP.add)
            S_cur = s_new

            # per-chunk epilogue: one sqrt for all 8 heads, then gate+transpose to oS
            nc.scalar.activation(osum8[:], osum8[:], AF.Sqrt, bias=epsg[:], scale=1.0 / DV)
            nc.vector.reciprocal(osum8[:], osum8[:])
            for h in range(HP):
                oute = dp3.tile([128, DV], F32, tag="oute")
                nc.vector.scalar_tensor_tensor(
                    oute[:], otA[:, DV * h:DV * h + DV], osum8[:, h:h + 1],
                    g_tok[:, VD_C * ci + DV * h:VD_C * ci + DV * h + DV],
                    op0=OP.mult, op1=OP.mult)
                pOt = cpst()
                nc.tensor.transpose(pOt[0:DV, 0:128], oute[:], idn[:])
                nc.scalar.copy(oS[0:DV, 1024 * h + 128 * ci:1024 * h + 128 * ci + 128],
                               pOt[0:DV, 0:128])

            # interleaved o_proj for this token block (Phase D work)
            if do_d:
                for dh in range(4):
                    pp = cpst()
                    for bb in range(HP):
                        wt = wp2.tile([DV, 512], BF16, tag="wo")
                        nc.sync.dma_start(
                            wt[:], wo_d[DV * bb:DV * bb + DV, 512 * dh:512 * dh + 512])
                        nc.tensor.matmul(
                            pp[:], oS[0:DV, 1024 * bb + 128 * ci:1024 * bb + 128 * ci + 128],
                            wt[:], start=(bb == 0), stop=(bb == HP - 1))
                    stg = pd.tile([128, 512], BF16, tag="s512")
                    nc.scalar.copy(stg[:], pp[:])
                    nc.sync.dma_start(
                        o_in[128 * ci:128 * ci + 128, 512 * dh:512 * dh + 512], stg[:])
                if ci % 2 == 1:
                    q0 = (ci // 2) * 256
                    nc.gpsimd.collective_compute(
                        "AllReduce", OP.add, ins=[o_in[q0:q0 + 256, :]],
                        outs=[o_out[q0:q0 + 256, :]], replica_groups=groups)

        for p in (psCh, psC, pd, wp2, spool, dp3, dp2, df32, dsl, dxp, dbf):
            p.release()
        qkvp.release()
        oSp.release()

        # ============ Phase E ============
        if "D" not in PHASES:
            raise _SkipRest()
        if "E" not in PHASES:
            raise _SkipRest()
        stE = tc.alloc_tile_pool(name="stE", bufs=3)
        psT = tc.alloc_tile_pool(name="psT", bufs=4, space="PSUM")
        ffT = hT
        for i in range(NTOK):
            xa = stE.tile([128, D], F32, tag="x2k")
            nc.sync.dma_start(xa[:], x_d[128 * i:128 * i + 128, :])
            ob = stE.tile([128, D], F32, tag="x2k")
            nc.gpsimd.dma_start(ob[:], o_out[128 * i:128 * i + 128, :])
            nc.vector.tensor_add(xa[:], xa[:], ob[:])
            nc.sync.dma_start(h2_scr[128 * i:128 * i + 128, :], xa[:])
            rcol = stE.tile([128, 1], F32, tag="rcol")
            nc.vector.tensor_mul(ob[:], xa[:], xa[:])
            nc.vector.tensor_reduce(rcol[:], ob[:], mybir.AxisListType.X, OP.add)
            nc.scalar.activation(rcol[:], rcol[:], AF.Sqrt, bias=eps1[:], scale=1.0 / D)
            nc.vector.reciprocal(rcol[:], rcol[:])
            xb = stE.tile([128, D], BF16, tag="xb")
            nc.vector.tensor_scalar_mul(xb[:], xa[:], rcol[:])
            for k in range(KT):
                pt = psT.tile([128, 128], BF16, tag="pt", name="ptE")
                nc.tensor.transpose(pt[:], xb[:, 128 * k:128 * k + 128], idh[:])
                if k % 2 == 0:
                    nc.scalar.copy(ffT[:, 1024 * k + 128 * i:1024 * k + 128 * i + 128], pt[:])
                else:
                    nc.vector.tensor_copy(ffT[:, 1024 * k + 128 * i:1024 * k + 128 * i + 128], pt[:])
        psT.release()
        psDE = tc.alloc_tile_pool(name="psDE", bufs=8, space="PSUM")
        pdE = tc.alloc_tile_pool(name="pdE", bufs=4)
        wpE = tc.alloc_tile_pool(name="wpE", bufs=7)

        def pst2(p=128, f=512):
            return psDE.tile([p, f], F32, tag="ps2", name="pst2")

        pgE = tc.alloc_tile_pool(name="pgE", bufs=1)
        mida = pgE.tile([128, 6 * 1024], BF16, tag="mida")
        pmid = tc.alloc_tile_pool(name="pmid", bufs=1)
        midb = pmid.tile([128, 5 * 1024], BF16, tag="midb")

        def mid_ap(m, off, ln):
            if m < 6:
                return mida[:, 1024 * m + off:1024 * m + off + ln]
            return midb[:, 1024 * (m - 6) + off:1024 * (m - 6) + off + ln]

        for mb in range(0, 12, 2):
            ms = [m for m in (mb, mb + 1) if m < 11]
            if not ms:
                break
            wid = 128 * len(ms)
            pus = {m: [pst2() for _ in range(2)] for m in ms}
            pvs = {m: [pst2() for _ in range(2)] for m in ms}
            for k in range(KT):
                wt1 = wp.tile([128, 256], BF16, tag="w")
                nc.sync.dma_start(
                    wt1[:, 0:wid],
                    w1_d[128 * k:128 * k + 128, 128 * mb:128 * mb + wid])
                wt3 = wp.tile([128, 256], BF16, tag="w")
                nc.sync.dma_start(
                    wt3[:, 0:wid],
                    w3_d[128 * k:128 * k + 128, 128 * mb:128 * mb + wid])
                for mi, m in enumerate(ms):
                    for n in range(2):
                        rhs = ffT[:, 1024 * k + 512 * n:1024 * k + 512 * n + 512]
                        nc.tensor.matmul(pus[m][n][:], wt1[:, 128 * mi:128 * mi + 128], rhs,
                                         start=(k == 0), stop=(k == KT - 1))
                        nc.tensor.matmul(pvs[m][n][:], wt3[:, 128 * mi:128 * mi + 128], rhs,
                                         start=(k == 0), stop=(k == KT - 1))
            for m in ms:
                for n in range(2):
                    u1s = pdE.tile([128, 512], F32, tag="s512")
                    nc.scalar.activation(u1s[:], pus[m][n][:], AF.Silu)
                    nc.vector.tensor_mul(mid_ap(m, 512 * n, 512), u1s[:], pvs[m][n][:])

        for dh in range(4):
            pps = [pst2() for _ in range(NTOK)]
            for mgrp in (range(0, 6), range(6, 11)):
                for m in mgrp:
                    wt = wpE.tile([128, 512], BF16, tag="w512")
                    nc.sync.dma_start(
                        wt[:], w2_d[128 * m:128 * m + 128, 512 * dh:512 * dh + 512])
                    for i in range(NTOK):
                        nc.tensor.matmul(pps[i][:], mid_ap(m, 128 * i, 128), wt[:],
                                         start=(m == 0), stop=(m == 10))
            for i in range(NTOK):
                h2t = pdE.tile([128, 512], F32, tag="s512")
                nc.sync.dma_start(h2t[:], h2_scr[128 * i:128 * i + 128, 512 * dh:512 * dh + 512])
                yst = pdE.tile([128, 512], F32, tag="s512")
                nc.vector.tensor_scalar_mul(yst[:], h2t[:], 0.25)
                nc.vector.tensor_add(yst[:], yst[:], pps[i][:])
                nc.sync.dma_start(y_d[128 * i:128 * i + 128, 512 * dh:512 * dh + 512], yst[:])

        for p in (pmid, pgE, wpE, pdE, stE, psDE, dram, wp, pg, big, cpool):
            p.release()
      except _SkipRest:
        zst = tc.alloc_tile_pool(name="zst", bufs=1)
        zt = zst.tile([128, 512], F32)
        nc.vector.memset(zt[:], 0.0)
        for i in range(NTOK):
            for dh in range(4):
                nc.sync.dma_start(y_d[128 * i:128 * i + 128, 512 * dh:512 * dh + 512], zt[:])
        zst.release()
        for pname in ("psCh", "psC", "pd", "wp2", "spool", "dp3", "dp2", "df32", "dsl",
                      "dxp", "dbf", "qkvp", "oSp", "pb", "psAB", "psDE", "stE", "dram", "wp",
                      "pg", "big", "cpool"):
            p = locals().get(pname)
            if p is not None:
                try:
                    p.release()
                except Exception:
                    pass

    nc.compile()
    return nc


def _shard(inputs):
    import ml_dtypes
    bf16 = ml_dtypes.bfloat16
    f32 = np.float32
    rms1 = np.asarray(inputs["rms1_w"], f32)
    rms2 = np.asarray(inputs["rms2_w"], f32)
    gn = np.asarray(inputs["gnorm_w"], f32)
    in_maps = []
    for c in range(8):
        g, m = c // 4, c % 4
        qs = slice(384 * m, 384 * m + 384)
        vs = slice(768 * m, 768 * m + 768)
        hs = slice(8 * m, 8 * m + 8)
        isl = slice(1408 * m, 1408 * m + 1408)

        def padqk(w):
            wp_ = np.zeros((D, QKP), f32)
            for h in range(8):
                wp_[:, 64 * h:64 * h + 48] = w[:, 48 * h:48 * h + 48]
            return wp_

        def padcw(w):
            cp = np.zeros((QKP, 4), f32)
            for h in range(8):
                cp[64 * h:64 * h + 48] = w[48 * h:48 * h + 48]
            return cp

        def padv(w):
            colpad = w.shape[0] == D
            out = np.zeros((D, VP) if colpad else (VP, w.shape[1]), f32)
            for h in range(8):
                if colpad:
                    out[:, 128 * h:128 * h + 96] = w[:, 96 * h:96 * h + 96]
                else:
                    out[128 * h:128 * h + 96] = w[96 * h:96 * h + 96]
            return out

        in_maps.append(dict(
            x=np.ascontiguousarray(np.asarray(inputs["x"], f32)[g]),
            wq=padqk(np.asarray(inputs["Wq"], f32)[:, qs] * rms1[:, None]).astype(bf16),
            wk=padqk(np.asarray(inputs["Wk"], f32)[:, qs] * rms1[:, None]).astype(bf16),
            wv=padv(np.asarray(inputs["Wv"], f32)[:, vs] * rms1[:, None]).astype(bf16),
            wg=np.ascontiguousarray(
                np.asarray(inputs["Wg"], f32)[:, vs] * rms1[:, None]).astype(bf16),
            wab=np.ascontiguousarray(np.concatenate(
                [np.asarray(inputs["Wa"], f32)[:, hs],
                 np.asarray(inputs["Wb"], f32)[:, hs]], 1) * rms1[:, None]),
            cq=padcw(np.asarray(inputs["conv_q_w"], f32)[qs]),
            ck=padcw(np.asarray(inputs["conv_k_w"], f32)[qs]),
            cv=padv(np.asarray(inputs["conv_v_w"], f32)[vs]),
            dtb=np.asarray(inputs["dt_bias"], f32)[hs].reshape(1, 8).copy(),
            nega=(-np.exp(np.asarray(inputs["A_log"], f32)[hs])).reshape(1, 8).copy(),
            wo=np.ascontiguousarray(
                np.asarray(inputs["Wo"], f32)[vs] * np.tile(gn, 8)[:, None]).astype(bf16),
            w1=np.ascontiguousarray(
                np.asarray(inputs["W1"], f32)[:, isl] * rms2[:, None]).astype(bf16),
            w3=np.ascontiguousarray(
                np.asarray(inputs["W3"], f32)[:, isl] * rms2[:, None]).astype(bf16),
            w2=np.ascontiguousarray(np.asarray(inputs["W2"], f32)[isl]).astype(bf16),
        ))
    return in_maps


def kernel(**inputs):
    if "nc" not in _cache:
        _cache["nc"] = _build(8)
    res = run_bass_kernel_spmd(_cache["nc"], _shard(inputs), list(range(8)))
    out = np.zeros((B, T, D), np.float32)
    for g in range(2):
        out[g] = sum(res.results[4 * g + m]["y"] for m in range(4))
    return out


# revision 72
# speedup vs baseline: 1.0695x; 1.0197x over previous
"""GatedDeltaNet block kernel for 8 Trainium2 cores (Bass/Tile).

Sharding: DP2 (batch) x TP4 (heads / MLP-inter). Core c: group g=c//4 runs
batch g; member m=c%4 owns heads [8m,8m+8), q/k cols [384m,..), v/g cols
[768m,..), INTER [1408m,..). One on-device AllReduce per 4-core group after
o_proj; final down-proj partials summed on the host.

Per-core dataflow:
  A: x -> rmsnorm (token-major) -> PE-transpose -> hT [2048,1024] f32r (SBUF)
  B: fp32r projections off hT; q/k feature-major (heads padded to 64 rows)
     -> conv+silu+l2norm -> bf16 SBUF qS/kS; v -> conv+silu -> bf16 SBUF vS;
     gate token-major direct (silu at evict) -> SBUF; a/b -> SBUF
  C: chunked gated delta rule (C=128, UT transform via log-doubling inverse)
     with bf16 matmul operands / fp32 psum+state; fused DVE ops; writes
     normed+gated o to DRAM (f32r)
  D: o_proj token-major partial -> AllReduce (group of 4)
  E: h2 = x + o_sum; rmsnorm -> transpose -> ffT (reuses hT slot); MLP f32r;
     y = W2_partial + 0.25*h2  (host sums the 4 partials per group)
"""
import sys
sys.path.insert(0, '/opt/trn_rl_repo')
import numpy as np

import concourse.bass as bass
import concourse.bacc as bacc
import concourse.mybir as mybir
import concourse.tile as tile
from concourse.bass_isa import ReduceOp
from concourse.bass_utils import run_bass_kernel_spmd

F32 = mybir.dt.float32
F32R = mybir.dt.float32r
BF16 = mybir.dt.bfloat16
AF = mybir.ActivationFunctionType
OP = mybir.AluOpType

B, T, D = 2, 1024, 2048
H, DK, DV = 32, 48, 96
HP = 8
QKP = 512
VD_C = 768
VP = 1024
INT_C = 1408
C = 128
NCHUNK = T // C
KT = D // 128
NTOK = T // 128

_cache = {}
import os
PHASES = os.environ.get("DN_PHASES", "ABCDE")
NCH = int(os.environ.get("DN_NCHUNK", str(T // C)))


class _SkipRest(Exception):
    pass


def _build(n_cores=8):
    groups = [[0, 1, 2, 3], [4, 5, 6, 7]] if n_cores == 8 else [[0]]
    nc = bacc.Bacc("TRN2", target_bir_lowering=False, debug=False, num_devices=n_cores)

    x_d = nc.dram_tensor("x", [T, D], F32, kind="ExternalInput")
    wq_d = nc.dram_tensor("wq", [D, QKP], BF16, kind="ExternalInput")
    wk_d = nc.dram_tensor("wk", [D, QKP], BF16, kind="ExternalInput")
    wv_d = nc.dram_tensor("wv", [D, VP], BF16, kind="ExternalInput")
    wg_d = nc.dram_tensor("wg", [D, VD_C], BF16, kind="ExternalInput")
    wab_d = nc.dram_tensor("wab", [D, 16], F32, kind="ExternalInput")
    cq_d = nc.dram_tensor("cq", [QKP, 4], F32, kind="ExternalInput")
    ck_d = nc.dram_tensor("ck", [QKP, 4], F32, kind="ExternalInput")
    cv_d = nc.dram_tensor("cv", [VP, 4], F32, kind="ExternalInput")
    dtb_d = nc.dram_tensor("dtb", [1, HP], F32, kind="ExternalInput")
    nega_d = nc.dram_tensor("nega", [1, HP], F32, kind="ExternalInput")
    wo_d = nc.dram_tensor("wo", [VD_C, D], BF16, kind="ExternalInput")
    w1_d = nc.dram_tensor("w1", [D, INT_C], BF16, kind="ExternalInput")
    w3_d = nc.dram_tensor("w3", [D, INT_C], BF16, kind="ExternalInput")
    w2_d = nc.dram_tensor("w2", [INT_C, D], BF16, kind="ExternalInput")
    y_d = nc.dram_tensor("y", [T, D], F32, kind="ExternalOutput")

    idn_c = nc.inline_tensor(np.eye(128, dtype=np.float32), "idn_c")
    ones = np.ones((128, 128), np.float32)
    cum_c = nc.inline_tensor(np.triu(ones).copy(), "cum_c")
    mst_c = nc.inline_tensor(np.triu(ones, 1).copy(), "mst_c")
    negl_c = nc.inline_tensor((np.tril(ones, -1) * -1e30).copy(), "negl_c")
    sel_np = np.zeros((HP, 512), np.float32)
    for j in range(4):
        sel_np[2 * j, 128 * j:128 * j + 48] = 1.0
        sel_np[2 * j + 1, 128 * j + 64:128 * j + 112] = 1.0
    sel_c = nc.inline_tensor(sel_np, "sel_c")
    on48_np = np.zeros((128, 2), np.float32)
    on48_np[0:48, 0] = 1.0
    on48_np[64:112, 1] = 1.0
    on48_c = nc.inline_tensor(on48_np, "on48_c")
    oneh_np = np.zeros((HP, HP * 128), np.float32)
    for h in range(HP):
        oneh_np[h, 128 * h:128 * h + 128] = 1.0
    oneh_c = nc.inline_tensor(oneh_np, "oneh_c")

    with tile.TileContext(nc) as tc:
      try:
        cpool = tc.alloc_tile_pool(name="consts", bufs=1)
        big = tc.alloc_tile_pool(name="big", bufs=1)
        pg = tc.alloc_tile_pool(name="pg", bufs=1)
        wp = tc.alloc_tile_pool(name="wp", bufs=6)
        oSp = tc.alloc_tile_pool(name="oSp", bufs=1)
        qkvp = tc.alloc_tile_pool(name="qkvp", bufs=1)
        dram = tc.alloc_tile_pool(name="dram", bufs=1, space="DRAM")

        idn = cpool.tile([128, 128], F32)
        idh = cpool.tile([128, 128], BF16)
        cum = cpool.tile([128, 128], F32)
        mst = cpool.tile([128, 128], BF16)
        negl = cpool.tile([128, 128], F32)
        sel = cpool.tile([HP, 512], F32)
        on48 = cpool.tile([128, 2], F32)
        oneh = cpool.tile([HP, HP * 128], F32)
        for t_, s_ in [(idn, idn_c), (cum, cum_c), (sel, sel_c),
                       (negl, negl_c), (oneh, oneh_c)]:
            nc.sync.dma_start(t_[:], s_[:])
        nc.gpsimd.dma_start(mst[:], mst_c[:])
        nc.sync.dma_start(on48[:], on48_c[:])
        nc.vector.tensor_copy(idh[:], idn[:])
        eps1 = cpool.tile([128, 1], F32)
        nc.vector.memset(eps1[:], 1e-5)
        epsq = cpool.tile([128, 1], F32)
        nc.vector.memset(epsq[:], 48e-6)
        epsk = cpool.tile([128, 1], F32)
        nc.vector.memset(epsk[:], 1e-6)
        epsg = cpool.tile([128, 1], F32)
        nc.vector.memset(epsg[:], 1e-5)
        dtb_r = cpool.tile([1, HP], F32)
        nega_r = cpool.tile([1, HP], F32)
        nc.sync.dma_start(dtb_r[:], dtb_d[:])
        nc.sync.dma_start(nega_r[:], nega_d[:])
        dtb_bc = cpool.tile([128, HP], F32)
        nega_bc = cpool.tile([128, HP], F32)
        nc.gpsimd.partition_broadcast(dtb_bc[:], dtb_r[:])
        nc.gpsimd.partition_broadcast(nega_bc[:], nega_r[:])
        cqw = cpool.tile([128, 16], F32)
        ckw = cpool.tile([128, 16], F32)
        cvw = cpool.tile([128, 32], F32)
        for j in range(4):
            nc.sync.dma_start(cqw[:, 4 * j:4 * j + 4], cq_d[128 * j:128 * j + 128, :])
            nc.sync.dma_start(ckw[:, 4 * j:4 * j + 4], ck_d[128 * j:128 * j + 128, :])
        for j in range(8):
            nc.sync.dma_start(cvw[:, 4 * j:4 * j + 4], cv_d[128 * j:128 * j + 128, :])
        wab_s = cpool.tile([128, 16 * KT], F32)
        for k in range(KT):
            nc.sync.dma_start(wab_s[:, 16 * k:16 * k + 16], wab_d[128 * k:128 * k + 128, :])
        ab_fm = cpool.tile([16, 1024], F32)

        hT = big.tile([128, KT * 1024], BF16)
        g_tok = pg.tile([128, NTOK * VD_C], BF16, tag="gtok")
        # o kept SBUF-resident, per-head 128-col token blocks (rows 0:96 used)
        oS = oSp.tile([128, HP * 1024], BF16, tag="oS")
        qS = qkvp.tile([128, 4 * 1024], BF16, tag="qS")
        kS = qkvp.tile([128, 4 * 1024], BF16, tag="kS")
        vS = qkvp.tile([128, 8 * 1024], BF16, tag="vS")
        nc.vector.memset(qS[:], 0.0)
        nc.vector.memset(kS[:], 0.0)

        o_in = dram.tile([T, D], BF16)
        o_out = dram.tile([T, D], BF16)
        h2_scr = dram.tile([T, D], F32)

        # ============ Phase A ============
        psAB = tc.alloc_tile_pool(name="psAB", bufs=8, space="PSUM")

        def pst(p=128, f=512):
            return psAB.tile([p, f], F32, tag="ps", name="pst")

        stA = tc.alloc_tile_pool(name="stA", bufs=3)
        stA2 = tc.alloc_tile_pool(name="stA2", bufs=4)
        for i in range(NTOK):
            xa = stA.tile([128, D], F32, tag="x2k")
            nc.sync.dma_start(xa[:], x_d[128 * i:128 * i + 128, :])
            sq = stA.tile([128, D], F32, tag="x2k")
            rcol = stA.tile([128, 1], F32, tag="rcol")
            nc.vector.tensor_mul(sq[:], xa[:], xa[:])
            nc.vector.tensor_reduce(rcol[:], sq[:], mybir.AxisListType.X, OP.add)
            nc.scalar.activation(rcol[:], rcol[:], AF.Sqrt, bias=eps1[:], scale=1.0 / D)
            nc.vector.reciprocal(rcol[:], rcol[:])
            nc.vector.tensor_scalar_mul(xa[:], xa[:], rcol[:])
            p_abi = pst(16, 128)
            for k in range(KT):
                pt = pst(128, 128)
                nc.tensor.transpose(pt[:], xa[:, 128 * k:128 * k + 128], idn[:])
                st32 = stA2.tile([128, 128], F32, tag="st32")
                if k % 2 == 0:
                    nc.scalar.copy(st32[:], pt[:])
                    nc.vector.tensor_copy(hT[:, 1024 * k + 128 * i:1024 * k + 128 * i + 128], st32[:])
                else:
                    nc.vector.tensor_copy(st32[:], pt[:])
                    nc.gpsimd.tensor_copy(hT[:, 1024 * k + 128 * i:1024 * k + 128 * i + 128], st32[:])
                nc.tensor.matmul(p_abi[:], wab_s[:, 16 * k:16 * k + 16], st32[:],
                                 start=(k == 0), stop=(k == KT - 1))
            nc.vector.tensor_copy(ab_fm[:, 128 * i:128 * i + 128], p_abi[:])
        stA2.release()
        stA.release()

        # ============ Phase B ============
        if "B" not in PHASES:
            raise _SkipRest()
        pb = tc.alloc_tile_pool(name="pb", bufs=6)

        def conv_silu(pre, cw, j, out_ap):
            acc = pb.tile([128, 1024], F32, tag="s1k")
            nc.scalar.activation(acc[:], pre[:], AF.Copy, scale=cw[:, 4 * j + 3:4 * j + 4])
            for s in (1, 2, 3):
                nc.vector.scalar_tensor_tensor(
                    acc[:, s:1024], pre[:, 0:1024 - s], cw[:, 4 * j + 3 - s:4 * j + 4 - s],
                    acc[:, s:1024], op0=OP.mult, op1=OP.add)
            nc.scalar.activation(out_ap, acc[:], AF.Silu)

        def qkv_pass(w_dram, outS, cw, eps_col, mult, do_l2, jbase, wcol0):
            # one pass: 4 feature blocks, k-outer, 8 psums, 1 wide DMA per k
            pps = [[pst() for n in range(2)] for j in range(4)]
            for k in range(KT):
                wt = wp.tile([128, 512], BF16, tag="wwide")
                nc.sync.dma_start(
                    wt[:], w_dram[128 * k:128 * k + 128, wcol0:wcol0 + 512])
                for j in range(4):
                    for n in range(2):
                        nc.tensor.matmul(
                            pps[j][n][:], wt[:, 128 * j:128 * j + 128],
                            hT[:, 1024 * k + 512 * n:1024 * k + 512 * n + 512],
                            start=(k == 0), stop=(k == KT - 1))
            for j in range(4):
                jj = jbase + j
                pre = pb.tile([128, 1024], F32, tag="s1k")
                for n in range(2):
                    nc.scalar.copy(pre[:, 512 * n:512 * n + 512], pps[j][n][:])
                if not do_l2:
                    conv_silu(pre, cw, jj, outS[:, 1024 * jj:1024 * jj + 1024])
                    continue
                blk = pb.tile([128, 1024], F32, tag="s1k")
                conv_silu(pre, cw, jj, blk[:])
                sq = pb.tile([128, 1024], F32, tag="s1k")
                nc.vector.tensor_mul(sq[:], blk[:], blk[:])
                for hh, rh in ((0, 0), (1, 64)):
                    srow = pb.tile([1, 1024], F32, tag="srow")
                    for n2 in range(2):
                        p_ssq = pst(1, 512)
                        nc.tensor.matmul(
                            p_ssq[:], on48[:, hh:hh + 1], sq[:, 512 * n2:512 * n2 + 512],
                            start=True, stop=True)
                        nc.scalar.activation(srow[:, 512 * n2:512 * n2 + 512], p_ssq[:],
                                             AF.Sqrt, bias=eps_col[0:1, :], scale=mult)
                    sbc = pb.tile([128, 1024], F32, tag="s1k")
                    nc.gpsimd.partition_broadcast(sbc[:], srow[:])
                    nc.vector.reciprocal(sbc[rh:rh + 48, :], sbc[rh:rh + 48, :])
                    nc.vector.tensor_mul(
                        outS[rh:rh + 48, 1024 * jj:1024 * jj + 1024],
                        blk[rh:rh + 48, :], sbc[rh:rh + 48, :])

        qkv_pass(wq_d, qS, cqw, epsq, 48.0, True, 0, 0)
        qkv_pass(wk_d, kS, ckw, epsk, 1.0, True, 0, 0)
        qkv_pass(wv_d, vS, cvw, None, None, False, 0, 0)
        qkv_pass(wv_d, vS, cvw, None, None, False, 4, 512)

        # gate token-major
        for n in range(2):
            pgs = [pst(128, 384) for _ in range(NTOK)]
            for k in range(KT):
                wt = wp.tile([128, 384], BF16, tag="wg384")
                nc.sync.dma_start(
                    wt[:], wg_d[128 * k:128 * k + 128, 384 * n:384 * n + 384])
                for i in range(NTOK):
                    nc.tensor.matmul(
                        pgs[i][:], hT[:, 1024 * k + 128 * i:1024 * k + 128 * i + 128], wt[:],
                        start=(k == 0), stop=(k == KT - 1))
            for i in range(NTOK):
                nc.scalar.activation(
                    g_tok[:, VD_C * i + 384 * n:VD_C * i + 384 * n + 384], pgs[i][:], AF.Silu)
        pb.release()
        psAB.release()

        # ============ Phase C (+ interleaved Phase D o_proj) ============
        if "C" not in PHASES:
            raise _SkipRest()
        dbf = tc.alloc_tile_pool(name="dbf", bufs=26)
        dxp = tc.alloc_tile_pool(name="dxp", bufs=10)
        dsl = tc.alloc_tile_pool(name="dsl", bufs=6)
        df32 = tc.alloc_tile_pool(name="df32", bufs=5)
        dp2 = tc.alloc_tile_pool(name="dp2", bufs=2)
        dp3 = tc.alloc_tile_pool(name="dp3", bufs=6)
        spool = tc.alloc_tile_pool(name="spool", bufs=2)
        wp2 = tc.alloc_tile_pool(name="wp2", bufs=9)
        pd = tc.alloc_tile_pool(name="pd", bufs=3)
        psC = tc.alloc_tile_pool(name="psC", bufs=5, space="PSUM")
        psCh = tc.alloc_tile_pool(name="psCh", bufs=3, space="PSUM")

        def cpst():
            return psC.tile([128, 512], F32, tag="c", name="cpst")

        def cpsth():
            return psCh.tile([128, 1024], BF16, tag="ch", name="cpsth")

        def b128():
            return dbf.tile([128, 128], BF16, tag="b128", name="b128")

        S_cur = spool.tile([128, 4 * DV], F32, tag="s", name="s")
        nc.vector.memset(S_cur[:], 0.0)
        do_d = "D" in PHASES

        for ci in range(NCH):
            cs = slice(128 * ci, 128 * ci + 128)
            # --- per-chunk decay/beta prep (f32); pPrep bank: ab@0, bcum@128, bT@256, ebc4@384
            pPrep = cpst()
            nc.tensor.transpose(pPrep[:, 0:16], ab_fm[:, cs], idn[0:16, 0:16])
            gt = dp2.tile([128, HP], F32, tag="gt")
            nc.vector.tensor_add(gt[:], pPrep[:, 0:HP], dtb_bc[:])
            nc.scalar.activation(gt[:], gt[:], AF.Exp)
            nc.vector.tensor_scalar_add(gt[:], gt[:], 1.0)
            nc.scalar.activation(gt[:], gt[:], AF.Ln)
            nc.vector.tensor_mul(gt[:], gt[:], nega_bc[:])
            beta = dp2.tile([128, HP], F32, tag="beta")
            nc.scalar.activation(beta[:], pPrep[:, HP:16], AF.Sigmoid)
            nbeta = dp2.tile([128, HP], F32, tag="nbeta")
            nc.vector.tensor_scalar_mul(nbeta[:], beta[:], -1.0)
            nc.tensor.matmul(pPrep[:, 128:128 + HP], cum[:], gt[:], start=True, stop=True)
            bcum = dp2.tile([128, HP], F32, tag="bcum")
            nc.vector.tensor_copy(bcum[:], pPrep[:, 128:128 + HP])
            lam = dp2.tile([128, HP], F32, tag="lam")
            nc.scalar.activation(lam[:], pPrep[:, 128:128 + HP], AF.Exp)
            nlam = dp2.tile([128, HP], F32, tag="nlam")
            nc.vector.tensor_scalar_mul(nlam[:], lam[:], -1.0)
            nc.tensor.transpose(pPrep[0:HP, 256:384], bcum[:], idn[:])
            b_fm = dp2.tile([HP, 128], F32, tag="bfm")
            nc.vector.tensor_copy(b_fm[:], pPrep[0:HP, 256:384])
            ebc = dp2.tile([HP, 1], F32, tag="ebc")
            nc.scalar.activation(ebc[:], b_fm[:, 127:128], AF.Exp)
            for j in range(4):
                nc.tensor.matmul(pPrep[:, 384 + j:385 + j], sel[:, 128 * j:128 * j + 128],
                                 ebc[:], start=True, stop=True)
            ebc4 = dp2.tile([128, 4], F32, tag="ebc4")
            nc.vector.tensor_copy(ebc4[:], pPrep[:, 384:388])

            # v token-major: pack all 8 heads' transposes in one bf16 bank
            pVt = cpsth()
            for h in range(HP):
                nc.tensor.transpose(pVt[:, DV * h:DV * h + DV],
                                    vS[0:DV, 1024 * h + 128 * ci:1024 * h + 128 * ci + 128],
                                    idh[0:DV, 0:DV])
            v_tok = dp2.tile([128, HP * DV], F32, tag="vtok")
            nc.vector.tensor_copy(v_tok[:], pVt[:, 0:HP * DV])

            # k token-major (for kw), packed
            pKt = cpsth()
            for j in range(4):
                nc.tensor.transpose(pKt[:, 128 * j:128 * j + 128],
                                    kS[:, 1024 * j + 128 * ci:1024 * j + 128 * ci + 128], idh[:])
            pXX = cpsth()

            S_bf = dsl.tile([128, 4 * DV], BF16, tag="sbf", name="sbf")
            nc.vector.tensor_copy(S_bf[:], S_cur[:])
            otA = dp2.tile([128, HP * DV], F32, tag="otA")
            osum8 = dp2.tile([128, HP], F32, tag="osum8")
            s_new = spool.tile([128, 4 * DV], F32, tag="s")

            # breadth-first over groups of 4 heads (2 j-blocks) to keep all
            # engines fed: per stage, 4 independent heads' ops back-to-back
            def kq_ap(S, h):
                j, hh = divmod(h, 2)
                rh = 64 * hh
                return S[rh:rh + 48, 1024 * j + 128 * ci:1024 * j + 128 * ci + 128]

            for grp in range(2):
                js = (2 * grp, 2 * grp + 1)
                hs = [2 * j + hh for j in js for hh in range(2)]
                pA, dte, dincl, wcol, dsm, xx, abar, xt = {}, {}, {}, {}, {}, {}, {}, {}
                for h in hs:
                    pA[h] = cpst()
                    nc.tensor.matmul(pA[h][:, 0:128], kq_ap(kS, h), kq_ap(kS, h),
                                     start=True, stop=True)
                    nc.tensor.matmul(pA[h][:, 128:256], kq_ap(kS, h), kq_ap(qS, h),
                                     start=True, stop=True)
                    nc.tensor.matmul(pA[h][:, 256:384], oneh[:, 128 * h:128 * h + 128],
                                     b_fm[:], start=True, stop=True)
                for h in hs:
                    dte[h] = df32.tile([128, 128], F32, tag="d32", name="dte")
                    nc.vector.scalar_tensor_tensor(
                        dte[h][:], pA[h][:, 256:384], bcum[:, h:h + 1], negl[:],
                        op0=OP.subtract, op1=OP.add)
                for h in hs:
                    dincl[h] = b128()
                    nc.scalar.activation(dincl[h][:], dte[h][:], AF.Exp)
                    wcol[h] = dp3.tile([128, 1], F32, tag="wcol", name="wcol")
                    nc.scalar.activation(wcol[h][:], dte[h][:, 127:128], AF.Exp)
                for h in hs:
                    dsm[h] = b128()
                    nc.gpsimd.tensor_mul(dsm[h][:], dincl[h][:], mst[:])
                for h in hs:
                    xx[h] = b128()
                    nc.vector.scalar_tensor_tensor(
                        xx[h][:], pA[h][:, 0:128], nbeta[:, h:h + 1], dsm[h][:],
                        op0=OP.mult, op1=OP.mult)
                for h in hs:
                    abar[h] = b128()
                    nc.vector.tensor_mul(abar[h][:], pA[h][:, 128:256], dincl[h][:])
                for h in hs:
                    nc.tensor.transpose(pXX[:, 128 * h:128 * h + 128], xx[h][:], idh[:])
                pm, xxa, xta = {}, {}, {}
                for idx, h in enumerate(hs):
                    xt[h] = b128()
                    if idx % 2 == 0:
                        nc.scalar.copy(xt[h][:], pXX[:, 128 * h:128 * h + 128])
                    else:
                        nc.vector.tensor_copy(xt[h][:], pXX[:, 128 * h:128 * h + 128])
                for h in hs:
                    t = b128()
                    nc.gpsimd.tensor_add(t[:], xx[h][:], idh[:])
                    pm[h] = t[:]
                    xxa[h], xta[h] = xx[h][:], xt[h][:]
                # UT doubling, 4 heads interleaved; pU bank: X^2@0, (X^2)^T@128, P@256
                for lvl in range(6):
                    last = lvl == 5
                    pU = {}
                    for h in hs:
                        pU[h] = cpst()
                        if not last:
                            nc.tensor.matmul(pU[h][:, 0:128], xta[h], xxa[h],
                                             start=True, stop=True)
                        nc.tensor.matmul(pU[h][:, 128:256], xxa[h], xta[h],
                                         start=True, stop=True)
                    xtn = {}
                    for idx, h in enumerate(hs):
                        if not last:
                            xp = dxp.tile([128, 256], BF16, tag="xpair", name="xp")
                            if (lvl + idx) % 2 == 0:
                                nc.vector.tensor_copy(xp[:], pU[h][:, 0:256])
                            else:
                                nc.scalar.copy(xp[:], pU[h][:, 0:256])
                            xtn[h] = xp[:, 128:256]
                            xxa[h] = xp[:, 0:128]
                        else:
                            t = b128()
                            nc.vector.tensor_copy(t[:], pU[h][:, 128:256])
                            xtn[h] = t[:]
                    for h in hs:
                        nc.tensor.matmul(pU[h][:, 256:384], idh[:], pm[h],
                                         start=True, stop=False)
                        nc.tensor.matmul(pU[h][:, 256:384], xtn[h], pm[h],
                                         start=False, stop=True)
                    for idx, h in enumerate(hs):
                        t = b128()
                        if (lvl + idx) % 2 == 0:
                            nc.scalar.copy(t[:], pU[h][:, 256:384])
                        else:
                            nc.vector.tensor_copy(t[:], pU[h][:, 256:384])
                        pm[h] = t[:]
                        xta[h] = xtn[h]
                # attention/state matmuls; pV bank: ks@0, w@128, oi@256, qs@384
                p_s = cpst()
                psc = {js[0]: 0, js[1]: 256}
                pV, r_, u_ = {}, {}, {}
                kw = {j: b128() for j in js}
                for h in hs:
                    j, hh = divmod(h, 2)
                    rh = 64 * hh
                    pV[h] = cpst()
                    nc.tensor.matmul(pV[h][:, 0:DV], kq_ap(kS, h),
                                     S_bf[rh:rh + 48, DV * j:DV * j + DV], start=True, stop=True)
                for h in hs:
                    r_[h] = dsl.tile([128, DV], BF16, tag="rr", name="rr")
                    nc.vector.scalar_tensor_tensor(
                        r_[h][:], pV[h][:, 0:DV], nlam[:, h:h + 1], v_tok[:, DV * h:DV * h + DV],
                        op0=OP.mult, op1=OP.add)
                for h in hs:
                    nc.tensor.matmul(pV[h][:, 128:128 + DV], pm[h], r_[h][:],
                                     start=True, stop=True)
                for h in hs:
                    u_[h] = dsl.tile([128, DV], BF16, tag="uu", name="uu")
                    nc.vector.tensor_scalar_mul(u_[h][:], pV[h][:, 128:128 + DV],
                                                beta[:, h:h + 1])
                for h in hs:
                    j, hh = divmod(h, 2)
                    rh = 64 * hh
                    nc.vector.tensor_scalar_mul(
                        kw[j][:, rh:rh + 48], pKt[:, 128 * j + rh:128 * j + rh + 48],
                        wcol[h][:])
                for h in hs:
                    j, hh = divmod(h, 2)
                    rh = 64 * hh
                    nc.tensor.matmul(pV[h][:, 256:256 + DV], abar[h][:], u_[h][:],
                                     start=True, stop=True)
                    nc.tensor.matmul(pV[h][:, 384:384 + DV], kq_ap(qS, h),
                                     S_bf[rh:rh + 48, DV * j:DV * j + DV], start=True, stop=True)
                    nc.tensor.matmul(p_s[rh:rh + 48, psc[j]:psc[j] + DV],
                                     kw[j][:, rh:rh + 48], u_[h][:], start=True, stop=True)
                for h in hs:
                    nc.vector.tensor_scalar_mul(
                        otA[:, DV * h:DV * h + DV], pV[h][:, 384:384 + DV], lam[:, h:h + 1])
                    nc.vector.tensor_add(
                        otA[:, DV * h:DV * h + DV], otA[:, DV * h:DV * h + DV],
                        pV[h][:, 256:256 + DV])
                for h in hs:
                    osq = dp3.tile([128, DV], F32, tag="osq", name="osq")
                    nc.vector.scalar_tensor_tensor(
                        osq[:], otA[:, DV * h:DV * h + DV], 1.0, otA[:, DV * h:DV * h + DV],
                        op0=OP.mult, op1=OP.mult, accum_out=osum8[:, h:h + 1])
                for j in js:
                    for rh2 in (0, 64):
                        nc.vector.scalar_tensor_tensor(
                            s_new[rh2:rh2 + 48, DV * j:DV * j + DV],
                            S_cur[rh2:rh2 + 48, DV * j:DV * j + DV],
                            ebc4[rh2:rh2 + 48, j:j + 1], p_s[rh2:rh2 + 48, psc[j]:psc[j] + DV],
                            op0=OP.mult, op1=OP.add)
            S_cur = s_new

            # per-chunk epilogue: one sqrt for all 8 heads, then gate+transpose to oS
            nc.scalar.activation(osum8[:], osum8[:], AF.Sqrt, bias=epsg[:], scale=1.0 / DV)
            nc.vector.reciprocal(osum8[:], osum8[:])
            for h in range(HP):
                oute = dp3.tile([128, DV], F32, tag="oute")
                nc.vector.scalar_tensor_tensor(
                    oute[:], otA[:, DV * h:DV * h + DV], osum8[:, h:h + 1],
                    g_tok[:, VD_C * ci + DV * h:VD_C * ci + DV * h + DV],
                    op0=OP.mult, op1=OP.mult)
                pOt = cpst()
                nc.tensor.transpose(pOt[0:DV, 0:128], oute[:], idn[:])
                nc.scalar.copy(oS[0:DV, 1024 * h + 128 * ci:1024 * h + 128 * ci + 128],
                               pOt[0:DV, 0:128])

            # interleaved o_proj for this token block (Phase D work)
            if do_d:
                for dh in range(4):
                    pp = cpst()
                    for bb in range(HP):
                        wt = wp2.tile([DV, 512], BF16, tag="wo")
                        nc.sync.dma_start(
                            wt[:], wo_d[DV * bb:DV * bb + DV, 512 * dh:512 * dh + 512])
                        nc.tensor.matmul(
                            pp[:], oS[0:DV, 1024 * bb + 128 * ci:1024 * bb + 128 * ci + 128],
                            wt[:], start=(bb == 0), stop=(bb == HP - 1))
                    stg = pd.tile([128, 512], BF16, tag="s512")
                    nc.scalar.copy(stg[:], pp[:])
                    nc.sync.dma_start(
                        o_in[128 * ci:128 * ci + 128, 512 * dh:512 * dh + 512], stg[:])
                if ci % 2 == 1:
                    q0 = (ci // 2) * 256
                    nc.gpsimd.collective_compute(
                        "AllReduce", OP.add, ins=[o_in[q0:q0 + 256, :]],
                        outs=[o_out[q0:q0 + 256, :]], replica_groups=groups)

        for p in (psCh, psC, pd, wp2, spool, dp3, dp2, df32, dsl, dxp, dbf):
            p.release()
        qkvp.release()
        oSp.release()

        # ============ Phase E ============
        if "D" not in PHASES:
            raise _SkipRest()
        if "E" not in PHASES:
            raise _SkipRest()
        stE = tc.alloc_tile_pool(name="stE", bufs=3)
        psT = tc.alloc_tile_pool(name="psT", bufs=4, space="PSUM")
        ffT = hT
        for i in range(NTOK):
            xa = stE.tile([128, D], F32, tag="x2k")
            nc.sync.dma_start(xa[:], x_d[128 * i:128 * i + 128, :])
            ob = stE.tile([128, D], F32, tag="x2k")
            nc.gpsimd.dma_start(ob[:], o_out[128 * i:128 * i + 128, :])
            nc.vector.tensor_add(xa[:], xa[:], ob[:])
            nc.sync.dma_start(h2_scr[128 * i:128 * i + 128, :], xa[:])
            rcol = stE.tile([128, 1], F32, tag="rcol")
            nc.vector.tensor_mul(ob[:], xa[:], xa[:])
            nc.vector.tensor_reduce(rcol[:], ob[:], mybir.AxisListType.X, OP.add)
            nc.scalar.activation(rcol[:], rcol[:], AF.Sqrt, bias=eps1[:], scale=1.0 / D)
            nc.vector.reciprocal(rcol[:], rcol[:])
            xb = stE.tile([128, D], BF16, tag="xb")
            nc.vector.tensor_scalar_mul(xb[:], xa[:], rcol[:])
            for k in range(KT):
                pt = psT.tile([128, 128], BF16, tag="pt", name="ptE")
                nc.tensor.transpose(pt[:], xb[:, 128 * k:128 * k + 128], idh[:])
                if k % 2 == 0:
                    nc.scalar.copy(ffT[:, 1024 * k + 128 * i:1024 * k + 128 * i + 128], pt[:])
                else:
                    nc.vector.tensor_copy(ffT[:, 1024 * k + 128 * i:1024 * k + 128 * i + 128], pt[:])
        psT.release()
        psDE = tc.alloc_tile_pool(name="psDE", bufs=8, space="PSUM")
        pdE = tc.alloc_tile_pool(name="pdE", bufs=4)
        wpE = tc.alloc_tile_pool(name="wpE", bufs=7)

        def pst2(p=128, f=512):
            return psDE.tile([p, f], F32, tag="ps2", name="pst2")

        pgE = tc.alloc_tile_pool(name="pgE", bufs=1)
        mida = pgE.tile([128, 6 * 1024], BF16, tag="mida")
        pmid = tc.alloc_tile_pool(name="pmid", bufs=1)
        midb = pmid.tile([128, 5 * 1024], BF16, tag="midb")

        def mid_ap(m, off, ln):
            if m < 6:
                return mida[:, 1024 * m + off:1024 * m + off + ln]
            return midb[:, 1024 * (m - 6) + off:1024 * (m - 6) + off + ln]

        for mb in range(0, 12, 2):
            ms = [m for m in (mb, mb + 1) if m < 11]
            if not ms:
                break
            wid = 128 * len(ms)
            pus = {m: [pst2() for _ in range(2)] for m in ms}
            pvs = {m: [pst2() for _ in range(2)] for m in ms}
            for k in range(KT):
                wt1 = wp.tile([128, 256], BF16, tag="w")
                nc.sync.dma_start(
                    wt1[:, 0:wid],
                    w1_d[128 * k:128 * k + 128, 128 * mb:128 * mb + wid])
                wt3 = wp.tile([128, 256], BF16, tag="w")
                nc.sync.dma_start(
                    wt3[:, 0:wid],
                    w3_d[128 * k:128 * k + 128, 128 * mb:128 * mb + wid])
                for mi, m in enumerate(ms):
                    for n in range(2):
                        rhs = ffT[:, 1024 * k + 512 * n:1024 * k + 512 * n + 512]
                        nc.tensor.matmul(pus[m][n][:], wt1[:, 128 * mi:128 * mi + 128], rhs,
                                         start=(k == 0), stop=(k == KT - 1))
                        nc.tensor.matmul(pvs[m][n][:], wt3[:, 128 * mi:128 * mi + 128], rhs,
                                         start=(k == 0), stop=(k == KT - 1))
            for m in ms:
                for n in range(2):
                    u1s = pdE.tile([128, 512], F32, tag="s512")
                    nc.scalar.activation(u1s[:], pus[m][n][:], AF.Silu)
                    nc.vector.tensor_mul(mid_ap(m, 512 * n, 512), u1s[:], pvs[m][n][:])

        for dh in range(4):
            pps = [pst2() for _ in range(NTOK)]
            for mgrp in (range(0, 6), range(6, 11)):
                for m in mgrp:
                    wt = wpE.tile([128, 512], BF16, tag="w512")
                    nc.sync.dma_start(
                        wt[:], w2_d[128 * m:128 * m + 128, 512 * dh:512 * dh + 512])
                    for i in range(NTOK):
                        nc.tensor.matmul(pps[i][:], mid_ap(m, 128 * i, 128), wt[:],
                                         start=(m == 0), stop=(m == 10))
            for i in range(NTOK):
                h2t = pdE.tile([128, 512], F32, tag="s512")
                nc.sync.dma_start(h2t[:], h2_scr[128 * i:128 * i + 128, 512 * dh:512 * dh + 512])
                yst = pdE.tile([128, 512], F32, tag="s512")
                nc.vector.tensor_scalar_mul(yst[:], h2t[:], 0.25)
                nc.vector.tensor_add(yst[:], yst[:], pps[i][:])
                nc.sync.dma_start(y_d[128 * i:128 * i + 128, 512 * dh:512 * dh + 512], yst[:])

        for p in (pmid, pgE, wpE, pdE, stE, psDE, dram, wp, pg, big, cpool):
            p.release()
      except _SkipRest:
        zst = tc.alloc_tile_pool(name="zst", bufs=1)
        zt = zst.tile([128, 512], F32)
        nc.vector.memset(zt[:], 0.0)
        for i in range(NTOK):
            for dh in range(4):
                nc.sync.dma_start(y_d[128 * i:128 * i + 128, 512 * dh:512 * dh + 512], zt[:])
        zst.release()
        for pname in ("psCh", "psC", "pd", "wp2", "spool", "dp3", "dp2", "df32", "dsl",
                      "dxp", "dbf", "qkvp", "oSp", "pb", "psAB", "psDE", "stE", "dram", "wp",
                      "pg", "big", "cpool"):
            p = locals().get(pname)
            if p is not None:
                try:
                    p.release()
                except Exception:
                    pass

    nc.compile()
    return nc


def _shard(inputs):
    import ml_dtypes
    bf16 = ml_dtypes.bfloat16
    f32 = np.float32
    rms1 = np.asarray(inputs["rms1_w"], f32)
    rms2 = np.asarray(inputs["rms2_w"], f32)
    gn = np.asarray(inputs["gnorm_w"], f32)
    in_maps = []
    for c in range(8):
        g, m = c // 4, c % 4
        qs = slice(384 * m, 384 * m + 384)
        vs = slice(768 * m, 768 * m + 768)
        hs = slice(8 * m, 8 * m + 8)
        isl = slice(1408 * m, 1408 * m + 1408)

        def padqk(w):
            wp_ = np.zeros((D, QKP), f32)
            for h in range(8):
                wp_[:, 64 * h:64 * h + 48] = w[:, 48 * h:48 * h + 48]
            return wp_

        def padcw(w):
            cp = np.zeros((QKP, 4), f32)
            for h in range(8):
                cp[64 * h:64 * h + 48] = w[48 * h:48 * h + 48]
            return cp

        def padv(w):
            colpad = w.shape[0] == D
            out = np.zeros((D, VP) if colpad else (VP, w.shape[1]), f32)
            for h in range(8):
                if colpad:
                    out[:, 128 * h:128 * h + 96] = w[:, 96 * h:96 * h + 96]
                else:
                    out[128 * h:128 * h + 96] = w[96 * h:96 * h + 96]
            return out

        in_maps.append(dict(
            x=np.ascontiguousarray(np.asarray(inputs["x"], f32)[g]),
            wq=padqk(np.asarray(inputs["Wq"], f32)[:, qs] * rms1[:, None]).astype(bf16),
            wk=padqk(np.asarray(inputs["Wk"], f32)[:, qs] * rms1[:, None]).astype(bf16),
            wv=padv(np.asarray(inputs["Wv"], f32)[:, vs] * rms1[:, None]).astype(bf16),
            wg=np.ascontiguousarray(
                np.asarray(inputs["Wg"], f32)[:, vs] * rms1[:, None]).astype(bf16),
            wab=np.ascontiguousarray(np.concatenate(
                [np.asarray(inputs["Wa"], f32)[:, hs],
                 np.asarray(inputs["Wb"], f32)[:, hs]], 1) * rms1[:, None]),
            cq=padcw(np.asarray(inputs["conv_q_w"], f32)[qs]),
            ck=padcw(np.asarray(inputs["conv_k_w"], f32)[qs]),
            cv=padv(np.asarray(inputs["conv_v_w"], f32)[vs]),
            dtb=np.asarray(inputs["dt_bias"], f32)[hs].reshape(1, 8).copy(),
            nega=(-np.exp(np.asarray(inputs["A_log"], f32)[hs])).reshape(1, 8).copy(),
            wo=np.ascontiguousarray(
                np.asarray(inputs["Wo"], f32)[vs] * np.tile(gn, 8)[:, None]).astype(bf16),
            w1=np.ascontiguousarray(
                np.asarray(inputs["W1"], f32)[:, isl] * rms2[:, None]).astype(bf16),
            w3=np.ascontiguousarray(
                np.asarray(inputs["W3"], f32)[:, isl] * rms2[:, None]).astype(bf16),
            w2=np.ascontiguousarray(np.asarray(inputs["W2"], f32)[isl]).astype(bf16),
        ))
    return in_maps


def kernel(**inputs):
    if "nc" not in _cache:
        _cache["nc"] = _build(8)
    res = run_bass_kernel_spmd(_cache["nc"], _shard(inputs), list(range(8)))
    out = np.zeros((B, T, D), np.float32)
    for g in range(2):
        out[g] = sum(res.results[4 * g + m]["y"] for m in range(4))
    return out


# revision 73
# speedup vs baseline: 1.0970x; 1.0258x over previous
"""GatedDeltaNet block kernel for 8 Trainium2 cores (Bass/Tile).

Sharding: DP2 (batch) x TP4 (heads / MLP-inter). Core c: group g=c//4 runs
batch g; member m=c%4 owns heads [8m,8m+8), q/k cols [384m,..), v/g cols
[768m,..), INTER [1408m,..). One on-device AllReduce per 4-core group after
o_proj; final down-proj partials summed on the host.

Per-core dataflow:
  A: x -> rmsnorm (token-major) -> PE-transpose -> hT [2048,1024] f32r (SBUF)
  B: fp32r projections off hT; q/k feature-major (heads padded to 64 rows)
     -> conv+silu+l2norm -> bf16 SBUF qS/kS; v -> conv+silu -> bf16 SBUF vS;
     gate token-major direct (silu at evict) -> SBUF; a/b -> SBUF
  C: chunked gated delta rule (C=128, UT transform via log-doubling inverse)
     with bf16 matmul operands / fp32 psum+state; fused DVE ops; writes
     normed+gated o to DRAM (f32r)
  D: o_proj token-major partial -> AllReduce (group of 4)
  E: h2 = x + o_sum; rmsnorm -> transpose -> ffT (reuses hT slot); MLP f32r;
     y = W2_partial + 0.25*h2  (host sums the 4 partials per group)
"""
import sys
sys.path.insert(0, '/opt/trn_rl_repo')
import numpy as np

import concourse.bass as bass
import concourse.bacc as bacc
import concourse.mybir as mybir
import concourse.tile as tile
from concourse.bass_isa import ReduceOp
from concourse.bass_utils import run_bass_kernel_spmd

F32 = mybir.dt.float32
F32R = mybir.dt.float32r
BF16 = mybir.dt.bfloat16
AF = mybir.ActivationFunctionType
OP = mybir.AluOpType

B, T, D = 2, 1024, 2048
H, DK, DV = 32, 48, 96
HP = 8
QKP = 512
VD_C = 768
VP = 1024
INT_C = 1408
C = 128
NCHUNK = T // C
KT = D // 128
NTOK = T // 128

_cache = {}
import os
PHASES = os.environ.get("DN_PHASES", "ABCDE")
NCH = int(os.environ.get("DN_NCHUNK", str(T // C)))


class _SkipRest(Exception):
    pass


def _build(n_cores=8):
    groups = [[0, 1, 2, 3], [4, 5, 6, 7]] if n_cores == 8 else [[0]]
    nc = bacc.Bacc("TRN2", target_bir_lowering=False, debug=False, num_devices=n_cores)

    x_d = nc.dram_tensor("x", [T, D], F32, kind="ExternalInput")
    wq_d = nc.dram_tensor("wq", [D, QKP], BF16, kind="ExternalInput")
    wk_d = nc.dram_tensor("wk", [D, QKP], BF16, kind="ExternalInput")
    wv_d = nc.dram_tensor("wv", [D, VP], BF16, kind="ExternalInput")
    wg_d = nc.dram_tensor("wg", [D, VD_C], BF16, kind="ExternalInput")
    wab_d = nc.dram_tensor("wab", [D, 16], F32, kind="ExternalInput")
    cq_d = nc.dram_tensor("cq", [QKP, 4], F32, kind="ExternalInput")
    ck_d = nc.dram_tensor("ck", [QKP, 4], F32, kind="ExternalInput")
    cv_d = nc.dram_tensor("cv", [VP, 4], F32, kind="ExternalInput")
    dtb_d = nc.dram_tensor("dtb", [1, HP], F32, kind="ExternalInput")
    nega_d = nc.dram_tensor("nega", [1, HP], F32, kind="ExternalInput")
    wo_d = nc.dram_tensor("wo", [VD_C, D], BF16, kind="ExternalInput")
    w1_d = nc.dram_tensor("w1", [D, INT_C], BF16, kind="ExternalInput")
    w3_d = nc.dram_tensor("w3", [D, INT_C], BF16, kind="ExternalInput")
    w2_d = nc.dram_tensor("w2", [INT_C, D], BF16, kind="ExternalInput")
    y_d = nc.dram_tensor("y", [T, D], F32, kind="ExternalOutput")

    idn_c = nc.inline_tensor(np.eye(128, dtype=np.float32), "idn_c")
    ones = np.ones((128, 128), np.float32)
    cum_c = nc.inline_tensor(np.triu(ones).copy(), "cum_c")
    mst_c = nc.inline_tensor(np.triu(ones, 1).copy(), "mst_c")
    negl_c = nc.inline_tensor((np.tril(ones, -1) * -1e30).copy(), "negl_c")
    sel_np = np.zeros((HP, 512), np.float32)
    for j in range(4):
        sel_np[2 * j, 128 * j:128 * j + 48] = 1.0
        sel_np[2 * j + 1, 128 * j + 64:128 * j + 112] = 1.0
    sel_c = nc.inline_tensor(sel_np, "sel_c")
    on48_np = np.zeros((128, 2), np.float32)
    on48_np[0:48, 0] = 1.0
    on48_np[64:112, 1] = 1.0
    on48_c = nc.inline_tensor(on48_np, "on48_c")
    oneh_np = np.zeros((HP, HP * 128), np.float32)
    for h in range(HP):
        oneh_np[h, 128 * h:128 * h + 128] = 1.0
    oneh_c = nc.inline_tensor(oneh_np, "oneh_c")

    with tile.TileContext(nc) as tc:
      try:
        cpool = tc.alloc_tile_pool(name="consts", bufs=1)
        big = tc.alloc_tile_pool(name="big", bufs=1)
        pg = tc.alloc_tile_pool(name="pg", bufs=1)
        wp = tc.alloc_tile_pool(name="wp", bufs=6)
        oSp = tc.alloc_tile_pool(name="oSp", bufs=1)
        qkvp = tc.alloc_tile_pool(name="qkvp", bufs=1)
        dram = tc.alloc_tile_pool(name="dram", bufs=1, space="DRAM")

        idn = cpool.tile([128, 128], F32)
        idh = cpool.tile([128, 128], BF16)
        cum = cpool.tile([128, 128], F32)
        mst = cpool.tile([128, 128], BF16)
        negl = cpool.tile([128, 128], F32)
        sel = cpool.tile([HP, 512], F32)
        on48 = cpool.tile([128, 2], F32)
        oneh = cpool.tile([HP, HP * 128], F32)
        for t_, s_ in [(idn, idn_c), (cum, cum_c), (sel, sel_c),
                       (negl, negl_c), (oneh, oneh_c)]:
            nc.sync.dma_start(t_[:], s_[:])
        nc.gpsimd.dma_start(mst[:], mst_c[:])
        nc.sync.dma_start(on48[:], on48_c[:])
        nc.vector.tensor_copy(idh[:], idn[:])
        eps1 = cpool.tile([128, 1], F32)
        nc.vector.memset(eps1[:], 1e-5)
        epsq = cpool.tile([128, 1], F32)
        nc.vector.memset(epsq[:], 48e-6)
        epsk = cpool.tile([128, 1], F32)
        nc.vector.memset(epsk[:], 1e-6)
        epsg = cpool.tile([128, 1], F32)
        nc.vector.memset(epsg[:], 1e-5)
        dtb_r = cpool.tile([1, HP], F32)
        nega_r = cpool.tile([1, HP], F32)
        nc.sync.dma_start(dtb_r[:], dtb_d[:])
        nc.sync.dma_start(nega_r[:], nega_d[:])
        dtb_bc = cpool.tile([128, HP], F32)
        nega_bc = cpool.tile([128, HP], F32)
        nc.gpsimd.partition_broadcast(dtb_bc[:], dtb_r[:])
        nc.gpsimd.partition_broadcast(nega_bc[:], nega_r[:])
        cqw = cpool.tile([128, 16], F32)
        ckw = cpool.tile([128, 16], F32)
        cvw = cpool.tile([128, 32], F32)
        for j in range(4):
            nc.sync.dma_start(cqw[:, 4 * j:4 * j + 4], cq_d[128 * j:128 * j + 128, :])
            nc.sync.dma_start(ckw[:, 4 * j:4 * j + 4], ck_d[128 * j:128 * j + 128, :])
        for j in range(8):
            nc.sync.dma_start(cvw[:, 4 * j:4 * j + 4], cv_d[128 * j:128 * j + 128, :])
        wab_s = cpool.tile([128, 16 * KT], F32)
        for k in range(KT):
            nc.sync.dma_start(wab_s[:, 16 * k:16 * k + 16], wab_d[128 * k:128 * k + 128, :])
        ab_fm = cpool.tile([16, 1024], F32)

        hT = big.tile([128, KT * 1024], BF16)
        g_tok = pg.tile([128, NTOK * VD_C], BF16, tag="gtok")
        # o kept SBUF-resident, per-head 128-col token blocks (rows 0:96 used)
        oS = oSp.tile([128, HP * 1024], BF16, tag="oS")
        qS = qkvp.tile([128, 4 * 1024], BF16, tag="qS")
        kS = qkvp.tile([128, 4 * 1024], BF16, tag="kS")
        vS = qkvp.tile([128, 8 * 1024], BF16, tag="vS")
        nc.vector.memset(qS[:], 0.0)
        nc.vector.memset(kS[:], 0.0)

        o_in = dram.tile([T, D], BF16)
        o_out = dram.tile([T, D], BF16)
        h2_scr = dram.tile([T, D], F32)

        # ============ Phase A ============
        psAB = tc.alloc_tile_pool(name="psAB", bufs=8, space="PSUM")

        def pst(p=128, f=512):
            return psAB.tile([p, f], F32, tag="ps", name="pst")

        stA = tc.alloc_tile_pool(name="stA", bufs=3)
        stA2 = tc.alloc_tile_pool(name="stA2", bufs=4)
        for i in range(NTOK):
            xa = stA.tile([128, D], F32, tag="x2k")
            nc.sync.dma_start(xa[:], x_d[128 * i:128 * i + 128, :])
            sq = stA.tile([128, D], F32, tag="x2k")
            rcol = stA.tile([128, 1], F32, tag="rcol")
            nc.vector.tensor_mul(sq[:], xa[:], xa[:])
            nc.vector.tensor_reduce(rcol[:], sq[:], mybir.AxisListType.X, OP.add)
            nc.scalar.activation(rcol[:], rcol[:], AF.Sqrt, bias=eps1[:], scale=1.0 / D)
            nc.vector.reciprocal(rcol[:], rcol[:])
            nc.vector.tensor_scalar_mul(xa[:], xa[:], rcol[:])
            p_abi = pst(16, 128)
            for k in range(KT):
                pt = pst(128, 128)
                nc.tensor.transpose(pt[:], xa[:, 128 * k:128 * k + 128], idn[:])
                st32 = stA2.tile([128, 128], F32, tag="st32")
                if k % 2 == 0:
                    nc.scalar.copy(st32[:], pt[:])
                    nc.vector.tensor_copy(hT[:, 1024 * k + 128 * i:1024 * k + 128 * i + 128], st32[:])
                else:
                    nc.vector.tensor_copy(st32[:], pt[:])
                    nc.gpsimd.tensor_copy(hT[:, 1024 * k + 128 * i:1024 * k + 128 * i + 128], st32[:])
                nc.tensor.matmul(p_abi[:], wab_s[:, 16 * k:16 * k + 16], st32[:],
                                 start=(k == 0), stop=(k == KT - 1))
            nc.vector.tensor_copy(ab_fm[:, 128 * i:128 * i + 128], p_abi[:])
        stA2.release()
        stA.release()

        # ============ Phase B ============
        if "B" not in PHASES:
            raise _SkipRest()
        pb = tc.alloc_tile_pool(name="pb", bufs=6)

        def conv_silu(pre, cw, j, out_ap):
            acc = pb.tile([128, 1024], F32, tag="s1k")
            nc.scalar.activation(acc[:], pre[:], AF.Copy, scale=cw[:, 4 * j + 3:4 * j + 4])
            for s in (1, 2, 3):
                nc.vector.scalar_tensor_tensor(
                    acc[:, s:1024], pre[:, 0:1024 - s], cw[:, 4 * j + 3 - s:4 * j + 4 - s],
                    acc[:, s:1024], op0=OP.mult, op1=OP.add)
            nc.scalar.activation(out_ap, acc[:], AF.Silu)

        def qkv_pass(w_dram, outS, cw, eps_col, mult, do_l2, jbase, wcol0):
            # one pass: 4 feature blocks, k-outer, 8 psums, 1 wide DMA per k
            pps = [[pst() for n in range(2)] for j in range(4)]
            for k in range(KT):
                wt = wp.tile([128, 512], BF16, tag="wwide")
                nc.sync.dma_start(
                    wt[:], w_dram[128 * k:128 * k + 128, wcol0:wcol0 + 512])
                for j in range(4):
                    for n in range(2):
                        nc.tensor.matmul(
                            pps[j][n][:], wt[:, 128 * j:128 * j + 128],
                            hT[:, 1024 * k + 512 * n:1024 * k + 512 * n + 512],
                            start=(k == 0), stop=(k == KT - 1))
            for j in range(4):
                jj = jbase + j
                pre = pb.tile([128, 1024], F32, tag="s1k")
                for n in range(2):
                    nc.scalar.copy(pre[:, 512 * n:512 * n + 512], pps[j][n][:])
                if not do_l2:
                    conv_silu(pre, cw, jj, outS[:, 1024 * jj:1024 * jj + 1024])
                    continue
                blk = pb.tile([128, 1024], F32, tag="s1k")
                conv_silu(pre, cw, jj, blk[:])
                sq = pb.tile([128, 1024], F32, tag="s1k")
                nc.vector.tensor_mul(sq[:], blk[:], blk[:])
                for hh, rh in ((0, 0), (1, 64)):
                    srow = pb.tile([1, 1024], F32, tag="srow")
                    for n2 in range(2):
                        p_ssq = pst(1, 512)
                        nc.tensor.matmul(
                            p_ssq[:], on48[:, hh:hh + 1], sq[:, 512 * n2:512 * n2 + 512],
                            start=True, stop=True)
                        nc.scalar.activation(srow[:, 512 * n2:512 * n2 + 512], p_ssq[:],
                                             AF.Sqrt, bias=eps_col[0:1, :], scale=mult)
                    sbc = pb.tile([128, 1024], F32, tag="s1k")
                    nc.gpsimd.partition_broadcast(sbc[:], srow[:])
                    nc.vector.reciprocal(sbc[rh:rh + 48, :], sbc[rh:rh + 48, :])
                    nc.vector.tensor_mul(
                        outS[rh:rh + 48, 1024 * jj:1024 * jj + 1024],
                        blk[rh:rh + 48, :], sbc[rh:rh + 48, :])

        qkv_pass(wq_d, qS, cqw, epsq, 48.0, True, 0, 0)
        qkv_pass(wk_d, kS, ckw, epsk, 1.0, True, 0, 0)
        qkv_pass(wv_d, vS, cvw, None, None, False, 0, 0)
        qkv_pass(wv_d, vS, cvw, None, None, False, 4, 512)

        # gate token-major
        for n in range(2):
            pgs = [pst(128, 384) for _ in range(NTOK)]
            for k in range(KT):
                wt = wp.tile([128, 384], BF16, tag="wg384")
                nc.sync.dma_start(
                    wt[:], wg_d[128 * k:128 * k + 128, 384 * n:384 * n + 384])
                for i in range(NTOK):
                    nc.tensor.matmul(
                        pgs[i][:], hT[:, 1024 * k + 128 * i:1024 * k + 128 * i + 128], wt[:],
                        start=(k == 0), stop=(k == KT - 1))
            for i in range(NTOK):
                nc.scalar.activation(
                    g_tok[:, VD_C * i + 384 * n:VD_C * i + 384 * n + 384], pgs[i][:], AF.Silu)
        pb.release()
        psAB.release()

        # ============ Phase C (+ interleaved Phase D o_proj) ============
        if "C" not in PHASES:
            raise _SkipRest()
        dbf = tc.alloc_tile_pool(name="dbf", bufs=26)
        dxp = tc.alloc_tile_pool(name="dxp", bufs=10)
        dsl = tc.alloc_tile_pool(name="dsl", bufs=6)
        df32 = tc.alloc_tile_pool(name="df32", bufs=5)
        dp2 = tc.alloc_tile_pool(name="dp2", bufs=2)
        dp3 = tc.alloc_tile_pool(name="dp3", bufs=6)
        spool = tc.alloc_tile_pool(name="spool", bufs=2)
        wp2 = tc.alloc_tile_pool(name="wp2", bufs=9)
        pd = tc.alloc_tile_pool(name="pd", bufs=3)
        psC = tc.alloc_tile_pool(name="psC", bufs=5, space="PSUM")
        psCh = tc.alloc_tile_pool(name="psCh", bufs=3, space="PSUM")

        def cpst():
            return psC.tile([128, 512], F32, tag="c", name="cpst")

        def cpsth():
            return psCh.tile([128, 1024], BF16, tag="ch", name="cpsth")

        def b128():
            return dbf.tile([128, 128], BF16, tag="b128", name="b128")

        S_cur = spool.tile([128, 4 * DV], F32, tag="s", name="s")
        nc.vector.memset(S_cur[:], 0.0)
        do_d = "D" in PHASES

        for ci in range(NCH):
            cs = slice(128 * ci, 128 * ci + 128)
            # --- per-chunk decay/beta prep (f32); pPrep bank: ab@0, bcum@128, bT@256, ebc4@384
            pPrep = cpst()
            nc.tensor.transpose(pPrep[:, 0:16], ab_fm[:, cs], idn[0:16, 0:16])
            gt = dp2.tile([128, HP], F32, tag="gt")
            nc.vector.tensor_add(gt[:], pPrep[:, 0:HP], dtb_bc[:])
            nc.scalar.activation(gt[:], gt[:], AF.Exp)
            nc.vector.tensor_scalar_add(gt[:], gt[:], 1.0)
            nc.scalar.activation(gt[:], gt[:], AF.Ln)
            nc.vector.tensor_mul(gt[:], gt[:], nega_bc[:])
            beta = dp2.tile([128, HP], F32, tag="beta")
            nc.scalar.activation(beta[:], pPrep[:, HP:16], AF.Sigmoid)
            nbeta = dp2.tile([128, HP], F32, tag="nbeta")
            nc.vector.tensor_scalar_mul(nbeta[:], beta[:], -1.0)
            nc.tensor.matmul(pPrep[:, 128:128 + HP], cum[:], gt[:], start=True, stop=True)
            bcum = dp2.tile([128, HP], F32, tag="bcum")
            nc.vector.tensor_copy(bcum[:], pPrep[:, 128:128 + HP])
            lam = dp2.tile([128, HP], F32, tag="lam")
            nc.scalar.activation(lam[:], pPrep[:, 128:128 + HP], AF.Exp)
            nlam = dp2.tile([128, HP], F32, tag="nlam")
            nc.vector.tensor_scalar_mul(nlam[:], lam[:], -1.0)
            nc.tensor.transpose(pPrep[0:HP, 256:384], bcum[:], idn[:])
            b_fm = dp2.tile([HP, 128], F32, tag="bfm")
            nc.vector.tensor_copy(b_fm[:], pPrep[0:HP, 256:384])
            ebc = dp2.tile([HP, 1], F32, tag="ebc")
            nc.scalar.activation(ebc[:], b_fm[:, 127:128], AF.Exp)
            for j in range(4):
                nc.tensor.matmul(pPrep[:, 384 + j:385 + j], sel[:, 128 * j:128 * j + 128],
                                 ebc[:], start=True, stop=True)
            ebc4 = dp2.tile([128, 4], F32, tag="ebc4")
            nc.vector.tensor_copy(ebc4[:], pPrep[:, 384:388])

            # v token-major: pack all 8 heads' transposes in one bf16 bank
            pVt = cpsth()
            for h in range(HP):
                nc.tensor.transpose(pVt[:, DV * h:DV * h + DV],
                                    vS[0:DV, 1024 * h + 128 * ci:1024 * h + 128 * ci + 128],
                                    idh[0:DV, 0:DV])
            v_tok = dp2.tile([128, HP * DV], F32, tag="vtok")
            nc.vector.tensor_copy(v_tok[:], pVt[:, 0:HP * DV])

            # k token-major (for kw), packed
            pKt = cpsth()
            for j in range(4):
                nc.tensor.transpose(pKt[:, 128 * j:128 * j + 128],
                                    kS[:, 1024 * j + 128 * ci:1024 * j + 128 * ci + 128], idh[:])
            pXX = cpsth()

            S_bf = dsl.tile([128, 4 * DV], BF16, tag="sbf", name="sbf")
            nc.vector.tensor_copy(S_bf[:], S_cur[:])
            otA = dp2.tile([128, HP * DV], F32, tag="otA")
            osum8 = dp2.tile([128, HP], F32, tag="osum8")
            s_new = spool.tile([128, 4 * DV], F32, tag="s")

            # breadth-first over groups of 4 heads (2 j-blocks) to keep all
            # engines fed: per stage, 4 independent heads' ops back-to-back
            def kq_ap(S, h):
                j, hh = divmod(h, 2)
                rh = 64 * hh
                return S[rh:rh + 48, 1024 * j + 128 * ci:1024 * j + 128 * ci + 128]

            for grp in range(2):
                js = (2 * grp, 2 * grp + 1)
                hs = [2 * j + hh for j in js for hh in range(2)]
                pA, dte, dincl, wcol, dsm, xx, abar, xt = {}, {}, {}, {}, {}, {}, {}, {}
                for h in hs:
                    pA[h] = cpst()
                    nc.tensor.matmul(pA[h][:, 0:128], kq_ap(kS, h), kq_ap(kS, h),
                                     start=True, stop=True)
                    nc.tensor.matmul(pA[h][:, 128:256], kq_ap(kS, h), kq_ap(qS, h),
                                     start=True, stop=True)
                    nc.tensor.matmul(pA[h][:, 256:384], oneh[:, 128 * h:128 * h + 128],
                                     b_fm[:], start=True, stop=True)
                for h in hs:
                    dte[h] = df32.tile([128, 128], F32, tag="d32", name="dte")
                    nc.vector.scalar_tensor_tensor(
                        dte[h][:], pA[h][:, 256:384], bcum[:, h:h + 1], negl[:],
                        op0=OP.subtract, op1=OP.add)
                for h in hs:
                    dincl[h] = b128()
                    nc.scalar.activation(dincl[h][:], dte[h][:], AF.Exp)
                    wcol[h] = dp3.tile([128, 1], F32, tag="wcol", name="wcol")
                    nc.scalar.activation(wcol[h][:], dte[h][:, 127:128], AF.Exp)
                for h in hs:
                    dsm[h] = b128()
                    nc.gpsimd.tensor_mul(dsm[h][:], dincl[h][:], mst[:])
                for h in hs:
                    xx[h] = b128()
                    nc.vector.scalar_tensor_tensor(
                        xx[h][:], pA[h][:, 0:128], nbeta[:, h:h + 1], dsm[h][:],
                        op0=OP.mult, op1=OP.mult)
                for h in hs:
                    abar[h] = b128()
                    nc.vector.tensor_mul(abar[h][:], pA[h][:, 128:256], dincl[h][:])
                for h in hs:
                    nc.tensor.transpose(pXX[:, 128 * h:128 * h + 128], xx[h][:], idh[:])
                pm, xxa, xta = {}, {}, {}
                for idx, h in enumerate(hs):
                    xt[h] = b128()
                    if idx % 2 == 0:
                        nc.scalar.copy(xt[h][:], pXX[:, 128 * h:128 * h + 128])
                    else:
                        nc.vector.tensor_copy(xt[h][:], pXX[:, 128 * h:128 * h + 128])
                for h in hs:
                    t = b128()
                    nc.gpsimd.tensor_add(t[:], xx[h][:], idh[:])
                    pm[h] = t[:]
                    xxa[h], xta[h] = xx[h][:], xt[h][:]
                # UT doubling, 4 heads interleaved; pU bank: X^2@0, (X^2)^T@128, P@256
                for lvl in range(5):
                    last = lvl == 4
                    pU = {}
                    for h in hs:
                        pU[h] = cpst()
                        if not last:
                            nc.tensor.matmul(pU[h][:, 0:128], xta[h], xxa[h],
                                             start=True, stop=True)
                        nc.tensor.matmul(pU[h][:, 128:256], xxa[h], xta[h],
                                         start=True, stop=True)
                    xtn = {}
                    for idx, h in enumerate(hs):
                        if not last:
                            xp = dxp.tile([128, 256], BF16, tag="xpair", name="xp")
                            if (lvl + idx) % 2 == 0:
                                nc.vector.tensor_copy(xp[:], pU[h][:, 0:256])
                            else:
                                nc.scalar.copy(xp[:], pU[h][:, 0:256])
                            xtn[h] = xp[:, 128:256]
                            xxa[h] = xp[:, 0:128]
                        else:
                            t = b128()
                            nc.vector.tensor_copy(t[:], pU[h][:, 128:256])
                            xtn[h] = t[:]
                    for h in hs:
                        nc.tensor.matmul(pU[h][:, 256:384], idh[:], pm[h],
                                         start=True, stop=False)
                        nc.tensor.matmul(pU[h][:, 256:384], xtn[h], pm[h],
                                         start=False, stop=True)
                    for idx, h in enumerate(hs):
                        t = b128()
                        if (lvl + idx) % 2 == 0:
                            nc.scalar.copy(t[:], pU[h][:, 256:384])
                        else:
                            nc.vector.tensor_copy(t[:], pU[h][:, 256:384])
                        pm[h] = t[:]
                        xta[h] = xtn[h]
                # attention/state matmuls; pV bank: ks@0, w@128, oi@256, qs@384
                p_s = cpst()
                psc = {js[0]: 0, js[1]: 256}
                pV, r_, u_ = {}, {}, {}
                kw = {j: b128() for j in js}
                for h in hs:
                    j, hh = divmod(h, 2)
                    rh = 64 * hh
                    pV[h] = cpst()
                    nc.tensor.matmul(pV[h][:, 0:DV], kq_ap(kS, h),
                                     S_bf[rh:rh + 48, DV * j:DV * j + DV], start=True, stop=True)
                for h in hs:
                    r_[h] = dsl.tile([128, DV], BF16, tag="rr", name="rr")
                    nc.vector.scalar_tensor_tensor(
                        r_[h][:], pV[h][:, 0:DV], nlam[:, h:h + 1], v_tok[:, DV * h:DV * h + DV],
                        op0=OP.mult, op1=OP.add)
                for h in hs:
                    nc.tensor.matmul(pV[h][:, 128:128 + DV], pm[h], r_[h][:],
                                     start=True, stop=True)
                for h in hs:
                    u_[h] = dsl.tile([128, DV], BF16, tag="uu", name="uu")
                    nc.vector.tensor_scalar_mul(u_[h][:], pV[h][:, 128:128 + DV],
                                                beta[:, h:h + 1])
                for h in hs:
                    j, hh = divmod(h, 2)
                    rh = 64 * hh
                    nc.vector.tensor_scalar_mul(
                        kw[j][:, rh:rh + 48], pKt[:, 128 * j + rh:128 * j + rh + 48],
                        wcol[h][:])
                for h in hs:
                    j, hh = divmod(h, 2)
                    rh = 64 * hh
                    nc.tensor.matmul(pV[h][:, 256:256 + DV], abar[h][:], u_[h][:],
                                     start=True, stop=True)
                    nc.tensor.matmul(pV[h][:, 384:384 + DV], kq_ap(qS, h),
                                     S_bf[rh:rh + 48, DV * j:DV * j + DV], start=True, stop=True)
                    nc.tensor.matmul(p_s[rh:rh + 48, psc[j]:psc[j] + DV],
                                     kw[j][:, rh:rh + 48], u_[h][:], start=True, stop=True)
                for h in hs:
                    nc.vector.tensor_scalar_mul(
                        otA[:, DV * h:DV * h + DV], pV[h][:, 384:384 + DV], lam[:, h:h + 1])
                    nc.vector.tensor_add(
                        otA[:, DV * h:DV * h + DV], otA[:, DV * h:DV * h + DV],
                        pV[h][:, 256:256 + DV])
                for h in hs:
                    osq = dp3.tile([128, DV], F32, tag="osq", name="osq")
                    nc.vector.scalar_tensor_tensor(
                        osq[:], otA[:, DV * h:DV * h + DV], 1.0, otA[:, DV * h:DV * h + DV],
                        op0=OP.mult, op1=OP.mult, accum_out=osum8[:, h:h + 1])
                for j in js:
                    for rh2 in (0, 64):
                        nc.vector.scalar_tensor_tensor(
                            s_new[rh2:rh2 + 48, DV * j:DV * j + DV],
                            S_cur[rh2:rh2 + 48, DV * j:DV * j + DV],
                            ebc4[rh2:rh2 + 48, j:j + 1], p_s[rh2:rh2 + 48, psc[j]:psc[j] + DV],
                            op0=OP.mult, op1=OP.add)
            S_cur = s_new

            # per-chunk epilogue: one sqrt for all 8 heads, then gate+transpose to oS
            nc.scalar.activation(osum8[:], osum8[:], AF.Sqrt, bias=epsg[:], scale=1.0 / DV)
            nc.vector.reciprocal(osum8[:], osum8[:])
            for h in range(HP):
                oute = dp3.tile([128, DV], F32, tag="oute")
                nc.vector.scalar_tensor_tensor(
                    oute[:], otA[:, DV * h:DV * h + DV], osum8[:, h:h + 1],
                    g_tok[:, VD_C * ci + DV * h:VD_C * ci + DV * h + DV],
                    op0=OP.mult, op1=OP.mult)
                pOt = cpst()
                nc.tensor.transpose(pOt[0:DV, 0:128], oute[:], idn[:])
                nc.scalar.copy(oS[0:DV, 1024 * h + 128 * ci:1024 * h + 128 * ci + 128],
                               pOt[0:DV, 0:128])

            # interleaved o_proj for this token block (Phase D work)
            if do_d:
                for dh in range(4):
                    pp = cpst()
                    for bb in range(HP):
                        wt = wp2.tile([DV, 512], BF16, tag="wo")
                        nc.sync.dma_start(
                            wt[:], wo_d[DV * bb:DV * bb + DV, 512 * dh:512 * dh + 512])
                        nc.tensor.matmul(
                            pp[:], oS[0:DV, 1024 * bb + 128 * ci:1024 * bb + 128 * ci + 128],
                            wt[:], start=(bb == 0), stop=(bb == HP - 1))
                    stg = pd.tile([128, 512], BF16, tag="s512")
                    nc.scalar.copy(stg[:], pp[:])
                    nc.sync.dma_start(
                        o_in[128 * ci:128 * ci + 128, 512 * dh:512 * dh + 512], stg[:])
                if ci % 2 == 1:
                    q0 = (ci // 2) * 256
                    nc.gpsimd.collective_compute(
                        "AllReduce", OP.add, ins=[o_in[q0:q0 + 256, :]],
                        outs=[o_out[q0:q0 + 256, :]], replica_groups=groups)

        for p in (psCh, psC, pd, wp2, spool, dp3, dp2, df32, dsl, dxp, dbf):
            p.release()
        qkvp.release()
        oSp.release()

        # ============ Phase E ============
        if "D" not in PHASES:
            raise _SkipRest()
        if "E" not in PHASES:
            raise _SkipRest()
        stE = tc.alloc_tile_pool(name="stE", bufs=3)
        psT = tc.alloc_tile_pool(name="psT", bufs=4, space="PSUM")
        ffT = hT
        for i in range(NTOK):
            xa = stE.tile([128, D], F32, tag="x2k")
            nc.sync.dma_start(xa[:], x_d[128 * i:128 * i + 128, :])
            ob = stE.tile([128, D], F32, tag="x2k")
            nc.gpsimd.dma_start(ob[:], o_out[128 * i:128 * i + 128, :])
            nc.vector.tensor_add(xa[:], xa[:], ob[:])
            nc.sync.dma_start(h2_scr[128 * i:128 * i + 128, :], xa[:])
            rcol = stE.tile([128, 1], F32, tag="rcol")
            nc.vector.tensor_mul(ob[:], xa[:], xa[:])
            nc.vector.tensor_reduce(rcol[:], ob[:], mybir.AxisListType.X, OP.add)
            nc.scalar.activation(rcol[:], rcol[:], AF.Sqrt, bias=eps1[:], scale=1.0 / D)
            nc.vector.reciprocal(rcol[:], rcol[:])
            xb = stE.tile([128, D], BF16, tag="xb")
            nc.vector.tensor_scalar_mul(xb[:], xa[:], rcol[:])
            for k in range(KT):
                pt = psT.tile([128, 128], BF16, tag="pt", name="ptE")
                nc.tensor.transpose(pt[:], xb[:, 128 * k:128 * k + 128], idh[:])
                if k % 2 == 0:
                    nc.scalar.copy(ffT[:, 1024 * k + 128 * i:1024 * k + 128 * i + 128], pt[:])
                else:
                    nc.vector.tensor_copy(ffT[:, 1024 * k + 128 * i:1024 * k + 128 * i + 128], pt[:])
        psT.release()
        psDE = tc.alloc_tile_pool(name="psDE", bufs=8, space="PSUM")
        pdE = tc.alloc_tile_pool(name="pdE", bufs=4)
        wpE = tc.alloc_tile_pool(name="wpE", bufs=7)

        def pst2(p=128, f=512):
            return psDE.tile([p, f], F32, tag="ps2", name="pst2")

        pgE = tc.alloc_tile_pool(name="pgE", bufs=1)
        mida = pgE.tile([128, 6 * 1024], BF16, tag="mida")
        pmid = tc.alloc_tile_pool(name="pmid", bufs=1)
        midb = pmid.tile([128, 5 * 1024], BF16, tag="midb")

        def mid_ap(m, off, ln):
            if m < 6:
                return mida[:, 1024 * m + off:1024 * m + off + ln]
            return midb[:, 1024 * (m - 6) + off:1024 * (m - 6) + off + ln]

        for mb in range(0, 12, 2):
            ms = [m for m in (mb, mb + 1) if m < 11]
            if not ms:
                break
            wid = 128 * len(ms)
            pus = {m: [pst2() for _ in range(2)] for m in ms}
            pvs = {m: [pst2() for _ in range(2)] for m in ms}
            for k in range(KT):
                wt1 = wp.tile([128, 256], BF16, tag="w")
                nc.sync.dma_start(
                    wt1[:, 0:wid],
                    w1_d[128 * k:128 * k + 128, 128 * mb:128 * mb + wid])
                wt3 = wp.tile([128, 256], BF16, tag="w")
                nc.sync.dma_start(
                    wt3[:, 0:wid],
                    w3_d[128 * k:128 * k + 128, 128 * mb:128 * mb + wid])
                for mi, m in enumerate(ms):
                    for n in range(2):
                        rhs = ffT[:, 1024 * k + 512 * n:1024 * k + 512 * n + 512]
                        nc.tensor.matmul(pus[m][n][:], wt1[:, 128 * mi:128 * mi + 128], rhs,
                                         start=(k == 0), stop=(k == KT - 1))
                        nc.tensor.matmul(pvs[m][n][:], wt3[:, 128 * mi:128 * mi + 128], rhs,
                                         start=(k == 0), stop=(k == KT - 1))
            for m in ms:
                for n in range(2):
                    u1s = pdE.tile([128, 512], F32, tag="s512")
                    nc.scalar.activation(u1s[:], pus[m][n][:], AF.Silu)
                    nc.vector.tensor_mul(mid_ap(m, 512 * n, 512), u1s[:], pvs[m][n][:])

        for dh in range(4):
            pps = [pst2() for _ in range(NTOK)]
            for mgrp in (range(0, 6), range(6, 11)):
                for m in mgrp:
                    wt = wpE.tile([128, 512], BF16, tag="w512")
                    nc.sync.dma_start(
                        wt[:], w2_d[128 * m:128 * m + 128, 512 * dh:512 * dh + 512])
                    for i in range(NTOK):
                        nc.tensor.matmul(pps[i][:], mid_ap(m, 128 * i, 128), wt[:],
                                         start=(m == 0), stop=(m == 10))
            for i in range(NTOK):
                h2t = pdE.tile([128, 512], F32, tag="s512")
                nc.sync.dma_start(h2t[:], h2_scr[128 * i:128 * i + 128, 512 * dh:512 * dh + 512])
                yst = pdE.tile([128, 512], F32, tag="s512")
                nc.vector.tensor_scalar_mul(yst[:], h2t[:], 0.25)
                nc.vector.tensor_add(yst[:], yst[:], pps[i][:])
                nc.sync.dma_start(y_d[128 * i:128 * i + 128, 512 * dh:512 * dh + 512], yst[:])

        for p in (pmid, pgE, wpE, pdE, stE, psDE, dram, wp, pg, big, cpool):
            p.release()
      except _SkipRest:
        zst = tc.alloc_tile_pool(name="zst", bufs=1)
        zt = zst.tile([128, 512], F32)
        nc.vector.memset(zt[:], 0.0)
        for i in range(NTOK):
            for dh in range(4):
                nc.sync.dma_start(y_d[128 * i:128 * i + 128, 512 * dh:512 * dh + 512], zt[:])
        zst.release()
        for pname in ("psCh", "psC", "pd", "wp2", "spool", "dp3", "dp2", "df32", "dsl",
                      "dxp", "dbf", "qkvp", "oSp", "pb", "psAB", "psDE", "stE", "dram", "wp",
                      "pg", "big", "cpool"):
            p = locals().get(pname)
            if p is not None:
                try:
                    p.release()
                except Exception:
                    pass

    nc.compile()
    return nc


def _shard(inputs):
    import ml_dtypes
    bf16 = ml_dtypes.bfloat16
    f32 = np.float32
    rms1 = np.asarray(inputs["rms1_w"], f32)
    rms2 = np.asarray(inputs["rms2_w"], f32)
    gn = np.asarray(inputs["gnorm_w"], f32)
    in_maps = []
    for c in range(8):
        g, m = c // 4, c % 4
        qs = slice(384 * m, 384 * m + 384)
        vs = slice(768 * m, 768 * m + 768)
        hs = slice(8 * m, 8 * m + 8)
        isl = slice(1408 * m, 1408 * m + 1408)

        def padqk(w):
            wp_ = np.zeros((D, QKP), f32)
            for h in range(8):
                wp_[:, 64 * h:64 * h + 48] = w[:, 48 * h:48 * h + 48]
            return wp_

        def padcw(w):
            cp = np.zeros((QKP, 4), f32)
            for h in range(8):
                cp[64 * h:64 * h + 48] = w[48 * h:48 * h + 48]
            return cp

        def padv(w):
            colpad = w.shape[0] == D
            out = np.zeros((D, VP) if colpad else (VP, w.shape[1]), f32)
            for h in range(8):
                if colpad:
                    out[:, 128 * h:128 * h + 96] = w[:, 96 * h:96 * h + 96]
                else:
                    out[128 * h:128 * h + 96] = w[96 * h:96 * h + 96]
            return out

        in_maps.append(dict(
            x=np.ascontiguousarray(np.asarray(inputs["x"], f32)[g]),
            wq=padqk(np.asarray(inputs["Wq"], f32)[:, qs] * rms1[:, None]).astype(bf16),
            wk=padqk(np.asarray(inputs["Wk"], f32)[:, qs] * rms1[:, None]).astype(bf16),
            wv=padv(np.asarray(inputs["Wv"], f32)[:, vs] * rms1[:, None]).astype(bf16),
            wg=np.ascontiguousarray(
                np.asarray(inputs["Wg"], f32)[:, vs] * rms1[:, None]).astype(bf16),
            wab=np.ascontiguousarray(np.concatenate(
                [np.asarray(inputs["Wa"], f32)[:, hs],
                 np.asarray(inputs["Wb"], f32)[:, hs]], 1) * rms1[:, None]),
            cq=padcw(np.asarray(inputs["conv_q_w"], f32)[qs]),
            ck=padcw(np.asarray(inputs["conv_k_w"], f32)[qs]),
            cv=padv(np.asarray(inputs["conv_v_w"], f32)[vs]),
            dtb=np.asarray(inputs["dt_bias"], f32)[hs].reshape(1, 8).copy(),
            nega=(-np.exp(np.asarray(inputs["A_log"], f32)[hs])).reshape(1, 8).copy(),
            wo=np.ascontiguousarray(
                np.asarray(inputs["Wo"], f32)[vs] * np.tile(gn, 8)[:, None]).astype(bf16),
            w1=np.ascontiguousarray(
                np.asarray(inputs["W1"], f32)[:, isl] * rms2[:, None]).astype(bf16),
            w3=np.ascontiguousarray(
                np.asarray(inputs["W3"], f32)[:, isl] * rms2[:, None]).astype(bf16),
            w2=np.ascontiguousarray(np.asarray(inputs["W2"], f32)[isl]).astype(bf16),
        ))
    return in_maps


def kernel(**inputs):
    if "nc" not in _cache:
        _cache["nc"] = _build(8)
    res = run_bass_kernel_spmd(_cache["nc"], _shard(inputs), list(range(8)))
    out = np.zeros((B, T, D), np.float32)
    for g in range(2):
        out[g] = sum(res.results[4 * g + m]["y"] for m in range(4))
    return out
